# revision 1
# baseline (speedup 1.0000x reference)
"""Trainium2 Bass kernel for a dense transformer block (DyT-norm causal attention + GELU MLP).

Sharding: 8 cores, SPMD single NEFF. Core c handles batch b=c//4 and query tokens
[qs*512:(qs+1)*512] with qs=c%4. Each core computes K/V projections for the full
sequence of its batch (replicated across the 4 cores of a batch), attention for
its query slice over all 16 heads, then projection + MLP on its token slice.
No collectives: outputs are disjoint token slices, gathered on the host.

Causal masking with a uniform NEFF: the host permutes each core's key/value token
order to [query-window | earlier | later]. KV blocks 0-3 are then always the
diagonal (static triangular mask constants), and the remaining blocks are handled
by a per-core additive bias column (0 = keep, -30000 = drop) applied inside the
softmax exp. Softmax is computed un-shifted (logits are small at init scale), and
the denominator is fused into the attention@V matmul via a ones-column on V.

Matmuls run in float32r (full PE rate at free dim 512) except attention
score/AV matmuls which use bf16 operands with fp32 PSUM accumulation.
"""

import sys
from contextlib import ExitStack

for _p in ('/opt/trn_rl_repo',):
    if _p not in sys.path:
        sys.path.insert(0, _p)

import numpy as np
import ml_dtypes

import concourse.bass as bass
import concourse.mybir as mybir
from concourse.bacc import Bacc
from concourse.bass_utils import run_bass_kernel_spmd
from concourse.tile import TileContext

C = 1024
H = 16
D = 64
FF = 4096
T = 2048
TQ = 512          # query tokens per core
NEG = -30000.0
F32 = mybir.dt.float32
F32R = mybir.dt.float32r
BF16 = mybir.dt.bfloat16
AF = mybir.ActivationFunctionType
ALU = mybir.AluOpType

_CACHE = {}


def _r128(dram_ap):
    """[(m*128), f] DRAM view -> [128, m, f]"""
    return dram_ap.rearrange("(m p) f -> p m f", p=128)


def _build(phases='ABCD'):
    nc = Bacc(trn_type='TRN2')

    # ---- DRAM I/O ----
    xT_d = nc.dram_tensor('xT', [C, T], F32, kind='ExternalInput')
    xqb_d = nc.dram_tensor('xqb', [C, TQ], F32, kind='ExternalInput')
    # Weights are host-pretiled to [128, mt, kt, 128] so each matmul group's
    # lhsT tiles arrive in ONE contiguous-per-partition DMA.
    wq_d = nc.dram_tensor('wq', [128, 8, 8, 128], F32R, kind='ExternalInput')
    wk_d = nc.dram_tensor('wk', [128, 8, 8, 128], F32R, kind='ExternalInput')
    wv_d = nc.dram_tensor('wv', [C, C], F32R, kind='ExternalInput')
    wproj_d = nc.dram_tensor('wproj', [128, 8, 8, 128], F32R, kind='ExternalInput')
    wfc_d = nc.dram_tensor('wfc', [128, 32, 8, 128], F32R, kind='ExternalInput')
    wfc2_d = nc.dram_tensor('wfc2', [128, 8, 32, 128], F32R, kind='ExternalInput')
    bq_d = nc.dram_tensor('bq', [128, 8], F32, kind='ExternalInput')
    bk_d = nc.dram_tensor('bk', [128, 8], F32, kind='ExternalInput')
    bv_d = nc.dram_tensor('bv', [128, C], F32, kind='ExternalInput')
    bfc_d = nc.dram_tensor('bfc', [128, 32], F32, kind='ExternalInput')
    bfc2_d = nc.dram_tensor('bfc2', [128, 8], F32, kind='ExternalInput')
    alpha_d = nc.dram_tensor('alpha_b', [128, 1], F32, kind='ExternalInput')
    gamma_d = nc.dram_tensor('gamma_c', [128, 8], F32, kind='ExternalInput')
    beta_d = nc.dram_tensor('beta_c', [128, 8], F32, kind='ExternalInput')
    mtri_d = nc.dram_tensor('mask_tri', [128, 4, TQ], F32, kind='ExternalInput')
    bcol_d = nc.dram_tensor('bias_cols', [128, 8], F32, kind='ExternalInput')
    ones_d = nc.dram_tensor('ones_bf', [128, 16], BF16, kind='ExternalInput')
    yT_d = nc.dram_tensor('yT', [C, TQ], F32, kind='ExternalOutput')

    with TileContext(nc) as tc, ExitStack() as top:
        cpool = top.enter_context(tc.tile_pool(name='const', bufs=1))

        def cload(shape, dt, dram, tag):
            t = cpool.tile(shape, dt, tag=tag)
            nc.gpsimd.dma_start(t[:], dram[:])
            return t

        alpha_t = cload([128, 1], F32, alpha_d, 'c_alpha')
        gamma_t = cload([128, 8], F32, gamma_d, 'c_gamma')
        beta_t = cload([128, 8], F32, beta_d, 'c_beta')
        bq_t = cload([128, 8], F32, bq_d, 'c_bq')
        bk_t = cload([128, 8], F32, bk_d, 'c_bk')
        bv_t = cload([128, C], F32, bv_d, 'c_bv')
        bfc_t = cload([128, 32], F32, bfc_d, 'c_bfc')
        bfc2_t = cload([128, 8], F32, bfc2_d, 'c_bfc2')
        bcol2_t = cload([128, 8], F32, bcol_d, 'c_bcol')
        ones_t = cload([128, 16], BF16, ones_d, 'c_ones')

        xT_r = _r128(xT_d[:])      # [128, 8, 2048]
        xqb_r = _r128(xqb_d[:])    # [128, 8, 512]
        yT_r = _r128(yT_d[:])      # [128, 8, 512]

        # attnT outlives kqv (written in B, read in C); pools pop LIFO so it
        # opens first and closes at TileContext exit. Tile created lazily at
        # first use (phase B) so it doesn't occupy SBUF during phase A.
        attnT_pool = top.enter_context(tc.tile_pool(name='attnT', bufs=1))

        # K/Q/V buffers live through phases A+B
        es_kqv = ExitStack()
        kqv = es_kqv.enter_context(tc.tile_pool(name='kqv', bufs=1))
        K_bf = kqv.tile([128, 8, T], BF16)            # K^T
        Q_bf = kqv.tile([128, 8, TQ], BF16)           # Q^T
        V_bf = kqv.tile([128, 16, H, D + 1], BF16)    # token-major V + ones col

        # ================= Phase A: DyT + QKV projections =================
        with (
            tc.tile_pool(name='hT_pool', bufs=1) as hpool,
            tc.tile_pool(name='stageA', bufs=2) as spool,
            tc.tile_pool(name='wA', bufs=3) as wpool,
            tc.tile_pool(name='wvA', bufs=1) as wvpool,
            tc.tile_pool(name='psA', bufs=4, space='PSUM') as psA,
        ):
            hT = hpool.tile([128, 8, T], F32R)
            # DyT with gamma/beta folded into the weights host-side:
            # hT = tanh(alpha * x), batched 4 kt-chunks per op.
            # nt-outer so K-proj's first (mt, nt=0) group unblocks early.
            for nt in range(4):
                for k4 in range(2):
                    xt = spool.tile([128, 4, TQ], F32, tag='xstage')
                    nc.sync.dma_start(
                        xt[:], xT_r[:, k4 * 4:(k4 + 1) * 4, nt * TQ:(nt + 1) * TQ])
                    nc.scalar.activation(
                        hT[:, k4 * 4:(k4 + 1) * 4, nt * TQ:(nt + 1) * TQ],
                        xt[:], AF.Tanh, scale=alpha_t[:, 0:1])

            wv_r = _r128(wv_d[:])

            # Q^T = wq^T @ hT[:, :512]  (+bq)
            for mt in range(8):
                wt = wpool.tile([128, 8, 128], F32R, tag='wkq')
                nc.sync.dma_start(wt[:], wq_d[:, mt])
                ps = psA.tile([128, TQ], F32)
                for kt in range(8):
                    nc.tensor.matmul(ps[:], wt[:, kt, :], hT[:, kt, 0:TQ],
                                     start=(kt == 0), stop=(kt == 7))
                nc.vector.tensor_scalar(Q_bf[:, mt, :], ps[:],
                                        bq_t[:, mt:mt + 1], None, ALU.add)

            # K^T = wk^T @ hT  (+bk)
            for mt in range(8):
                wt = wpool.tile([128, 8, 128], F32R, tag='wkq')
                nc.sync.dma_start(wt[:], wk_d[:, mt])
                for nt in range(4):
                    ps = psA.tile([128, TQ], F32)
                    for kt in range(8):
                        nc.tensor.matmul(ps[:], wt[:, kt, :], hT[:, kt, nt * TQ:(nt + 1) * TQ],
                                         start=(kt == 0), stop=(kt == 7))
                    nc.vector.tensor_scalar(K_bf[:, mt, nt * TQ:(nt + 1) * TQ],
                                            ps[:], bk_t[:, mt:mt + 1], None, ALU.add)

            # V = hT^T @ wv (token-major) (+bv), into [128, kvb, head, 65] with ones col
            for n2 in range(2):
                wvt = wvpool.tile([128, 8, TQ], F32R, tag='wv')
                nc.sync.dma_start(wvt[:], wv_r[:, :, n2 * TQ:(n2 + 1) * TQ])
                for kvb in range(16):
                    ps = psA.tile([128, TQ], F32)
                    for kt in range(8):
                        nc.tensor.matmul(ps[:], hT[:, kt, kvb * 128:(kvb + 1) * 128],
                                         wvt[:, kt, :],
                                         start=(kt == 0), stop=(kt == 7))
                    bvb = bv_t[:, n2 * TQ:(n2 + 1) * TQ].rearrange(
                        "p (h d) -> p h d", d=D)
                    nc.vector.tensor_tensor(
                        V_bf[:, kvb, n2 * 8:(n2 + 1) * 8, 0:D],
                        ps[:].rearrange("p (h d) -> p h d", d=D),
                        bvb, ALU.add)
            for kvb in range(16):
                nc.vector.tensor_copy(V_bf[:, kvb, :, D], ones_t[:, :])

        # ================= Phase B: attention =================
        with (
            tc.tile_pool(name='pB', bufs=8) as pbpool,
            tc.tile_pool(name='mtriB', bufs=1) as mtpool,
            tc.tile_pool(name='psS', bufs=3, space='PSUM') as psS,
            tc.tile_pool(name='psO', bufs=2, space='PSUM') as psO,
        ):
            mtri_t = mtpool.tile([128, 4, TQ], F32)
            nc.gpsimd.dma_start(mtri_t[:], mtri_d[:])
            attnT = attnT_pool.tile([128, 8, TQ], F32R)
            for h in range(H if 'B' in phases else 0):
                hb = (h % 2) * 64
                hc = h // 2
                po = psO.tile([65, TQ], F32, tag='po')
                for kv2 in range(8):
                    # two kv blocks share one PSUM tile so exp runs [128, 1024]
                    ps = psS.tile([128, 2, TQ], F32, tag='score')
                    pt = pbpool.tile([128, 2, TQ], BF16, tag='probs')
                    for j in range(2):
                        kvb = kv2 * 2 + j
                        nc.tensor.matmul(ps[:, j, :],
                                         K_bf[hb:hb + 64, hc, kvb * 128:(kvb + 1) * 128],
                                         Q_bf[hb:hb + 64, hc, :],
                                         start=True, stop=True)
                        if kvb < 4:
                            nc.vector.tensor_tensor(ps[:, j, :], ps[:, j, :],
                                                    mtri_t[:, kvb, :], ALU.add)
                    nc.scalar.activation(
                        pt[:], ps[:], AF.Exp,
                        bias=bcol2_t[:, kv2:kv2 + 1], scale=0.125)
                    for j in range(2):
                        kvb = kv2 * 2 + j
                        nc.tensor.matmul(po[:], V_bf[:, kvb, h, :], pt[:, j, :],
                                         start=(kvb == 0), stop=(kvb == 15))
                rec = pbpool.tile([1, TQ], F32, tag='recip')
                nc.vector.reciprocal(rec[:], po[64:65, :])
                rec64 = pbpool.tile([64, TQ], F32, tag='recip64')
                nc.gpsimd.partition_broadcast(rec64[:], rec[0:1, :])
                nc.vector.tensor_tensor(attnT[hb:hb + 64, hc, :], po[0:64, :],
                                        rec64[:], ALU.mult)
        es_kqv.close()

        # x2T/h2T live through phases C+D
        es_mlp = ExitStack()
        mpool = es_mlp.enter_context(tc.tile_pool(name='mlp', bufs=1))
        x2T = mpool.tile([128, 8, TQ], F32)
        h2T = mpool.tile([128, 8, TQ], F32R)

        # ======== Phases C+D in one scope (wfc DMAs prefetch during proj) ====
        with (
            tc.tile_pool(name='stageC', bufs=3) as scpool,
            tc.tile_pool(name='xqbC', bufs=1) as xqpool,
            tc.tile_pool(name='wC', bufs=3) as wcpool,
            tc.tile_pool(name='gT_pool', bufs=1) as gpool,
            tc.tile_pool(name='wD', bufs=3) as wdpool,
            tc.tile_pool(name='wD2', bufs=2) as wd2pool,
            tc.tile_pool(name='psC', bufs=4, space='PSUM') as psC,
        ):
            xqb_t = xqpool.tile([128, 8, TQ], F32)
            nc.gpsimd.dma_start(xqb_t[:], xqb_r[:])
            for mt in range(8 if 'C' in phases else 0):
                wt = wcpool.tile([128, 8, 128], F32R, tag='wproj')
                nc.sync.dma_start(wt[:], wproj_d[:, mt])
                ps = psC.tile([128, TQ], F32)
                for kt in range(8):
                    nc.tensor.matmul(ps[:], wt[:, kt, :], attnT[:, kt, :],
                                     start=(kt == 0), stop=(kt == 7))
                nc.vector.tensor_tensor(x2T[:, mt, :], ps[:], xqb_t[:, mt, :], ALU.add)
                nc.scalar.activation(h2T[:, mt, :], x2T[:, mt, :], AF.Tanh,
                                     scale=alpha_t[:, 0:1])

            # ================= Phase D: MLP =================
            sdpool, psD = scpool, psC
            gT = gpool.tile([128, 32, TQ], F32R)
            for mt in range(32 if 'D' in phases else 0):
                wt = wdpool.tile([128, 8, 128], F32R, tag='wfc')
                nc.sync.dma_start(wt[:], wfc_d[:, mt])
                ps = psD.tile([128, TQ], F32)
                for kt in range(8):
                    nc.tensor.matmul(ps[:], wt[:, kt, :], h2T[:, kt, :],
                                     start=(kt == 0), stop=(kt == 7))
                nc.scalar.activation(gT[:, mt, :], ps[:], AF.Gelu,
                                     bias=bfc_t[:, mt:mt + 1])

            for mt in range(8 if 'D' in phases else 0):
                wt = wd2pool.tile([128, 32, 128], F32R, tag='wfc2')
                nc.sync.dma_start(wt[:], wfc2_d[:, mt])
                ps = psD.tile([128, TQ], F32)
                for kt in range(32):
                    nc.tensor.matmul(ps[:], wt[:, kt, :], gT[:, kt, :],
                                     start=(kt == 0), stop=(kt == 31))
                tmp = sdpool.tile([128, TQ], F32, tag='bias2')
                nc.vector.tensor_scalar(tmp[:], ps[:], bfc2_t[:, mt:mt + 1], None, ALU.add)
                yt = sdpool.tile([128, TQ], F32, tag='yout')
                nc.vector.tensor_tensor(yt[:], tmp[:], x2T[:, mt, :], ALU.add)
                nc.sync.dma_start(yT_r[:, mt, :], yt[:])
        es_mlp.close()

    nc.finalize()
    return nc


def _prep_inputs(x, alpha, gamma, beta, w_attn, b_attn, w_proj, b_proj,
                 w_fc, b_fc, w_fc2, b_fc2):
    f = np.float32

    def tile_w(w, n_mt):
        # [K, M] -> [128, mt, kt, 128]: element [p, mt, kt, c] = w[kt*128+p, mt*128+c]
        kk, mm = w.shape
        return np.ascontiguousarray(
            np.asarray(w, f).reshape(kk // 128, 128, n_mt, 128).transpose(1, 2, 0, 3))

    # Fold DyT's gamma/beta into the consuming weights:
    #   w.T @ (g*t + b) = (g[:,None]*w).T @ t + (w.T @ b)
    g64 = np.asarray(gamma, np.float64)
    b64 = np.asarray(beta, np.float64)
    w64 = np.asarray(w_attn, np.float64)
    wfc64 = np.asarray(w_fc, np.float64)
    wq64, wk64, wv64 = w64[:, :C], w64[:, C:2 * C], w64[:, 2 * C:]
    bq_e = np.asarray(b_attn[:C], np.float64) + wq64.T @ b64
    bk_e = np.asarray(b_attn[C:2 * C], np.float64) + wk64.T @ b64
    bv_e = np.asarray(b_attn[2 * C:], np.float64) + wv64.T @ b64
    bfc_e = np.asarray(b_fc, np.float64) + wfc64.T @ b64

    wq = tile_w(wq64 * g64[:, None], 8)
    wk = tile_w(wk64 * g64[:, None], 8)
    wv = np.ascontiguousarray(wv64 * g64[:, None], f)
    bq = np.ascontiguousarray(bq_e.reshape(8, 128).T, f)
    bk = np.ascontiguousarray(bk_e.reshape(8, 128).T, f)
    bv = np.ascontiguousarray(np.tile(bv_e.reshape(1, C), (128, 1)), f)
    bfc = np.ascontiguousarray(bfc_e.reshape(32, 128).T, f)
    bfc2 = np.ascontiguousarray(b_fc2.reshape(8, 128).T, f)
    alpha_b = np.full((128, 1), float(np.asarray(alpha).reshape(-1)[0]), f)
    gamma_c = np.ascontiguousarray(np.asarray(gamma, f).reshape(8, 128).T, f)
    beta_c = np.ascontiguousarray(np.asarray(beta, f).reshape(8, 128).T, f)
    r = np.arange(128)[:, None, None]
    tt = np.arange(4)[None, :, None]
    p = np.arange(TQ)[None, None, :]
    mask_tri = np.where(tt * 128 + r <= p, 0.0, NEG).astype(f)
    ones_bf = np.ones((128, 16), ml_dtypes.bfloat16)

    shared = dict(wq=wq, wk=wk, wv=wv, wproj=tile_w(w_proj, 8),
                  wfc=tile_w(wfc64 * g64[:, None], 32),
                  wfc2=tile_w(w_fc2, 8),
                  bq=bq, bk=bk, bv=bv, bfc=bfc, bfc2=bfc2,
                  alpha_b=alpha_b, gamma_c=gamma_c, beta_c=beta_c,
                  mask_tri=mask_tri, ones_bf=ones_bf)

    in_maps = []
    for c in range(8):
        b, qs = c // 4, c % 4
        perm = np.concatenate([np.arange(qs * TQ, (qs + 1) * TQ),
                               np.arange(0, qs * TQ),
                               np.arange((qs + 1) * TQ, T)])
        xT = np.ascontiguousarray(np.asarray(x[b], f).T[:, perm])
        xqb = np.ascontiguousarray(np.asarray(x[b, qs * TQ:(qs + 1) * TQ], f).T
                                   + np.asarray(b_proj, f)[:, None])
        bias_cols = np.zeros((128, 8), f)
        bias_cols[:, 2 + 2 * qs:] = NEG
        in_maps.append(dict(shared, xT=xT, xqb=xqb, bias_cols=bias_cols))
    return in_maps


def kernel(**inputs):
    if 'nc' not in _CACHE:
        _CACHE['nc'] = _build()
    nc = _CACHE['nc']
    in_maps = _prep_inputs(**inputs)
    res = run_bass_kernel_spmd(nc, in_maps, core_ids=list(range(8)))
    out = np.zeros((2, T, C), np.float32)
    for c in range(8):
        b, qs = c // 4, c % 4
        out[b, qs * TQ:(qs + 1) * TQ, :] = res.results[c]['yT'].T
    return out



# revision 53
# speedup vs baseline: 2.4511x; 2.4511x over previous
"""Trainium2 Bass kernel for a dense transformer block (DyT-norm causal attention + GELU MLP).

Sharding: 8 cores, SPMD single NEFF, no collectives. Core c handles batch b=c//4
and query tokens [qs*512:(qs+1)*512] with qs=c%4. Each core computes K/V for the
full sequence of its batch, attention for its query slice over all 16 heads, then
projection + MLP on its token slice. Outputs are disjoint; host gathers.

All large matmuls run as fp8e4m3 DoubleRow (2 contraction sub-tiles per
instruction, 0.5 PE cycles/row). Weights are host-scaled by 32 and quantized to
fp8; descales fold into PSUM-evacuation ops and activation scales. wq/wk columns
are host-permuted so Q^T/K^T land in SBUF as [32, 2, *] per head, letting the
64-deep score contraction also use DoubleRow. The attention@V matmul pairs two
whole kv blocks per DoubleRow instruction (equivalent to 2-step PSUM
accumulation). Softmax is unshifted exp with the denominator fused via a
ones-column on V; the V bias is folded through w_proj into the residual.

Causal masking: host permutes each core's key token order to [query-window |
earlier | later]. KV blocks 0-3 are the diagonal (additive -8e6 triangular mask
constants on the raw psum scores), remaining block-pairs use a per-core additive
bias column (0 / -30000) inside the softmax exp.
"""

import sys
from contextlib import ExitStack

for _p in ('/opt/trn_rl_repo',):
    if _p not in sys.path:
        sys.path.insert(0, _p)

import numpy as np
import ml_dtypes

import concourse.bass as bass
import concourse.mybir as mybir
from concourse.bacc import Bacc
from concourse.bass_utils import run_bass_kernel_spmd
from concourse.tile import TileContext

C = 1024
H = 16
D = 64
FF = 4096
T = 2048
TQ = 512          # query tokens per core
NEG = -30000.0    # exp bias-column mask (applied post-scale)
NEGM = -8.0e6     # additive score mask in raw psum units (pre 1/8192 scale)
SW = 32.0         # fp8 weight scale
F32 = mybir.dt.float32
BF16 = mybir.dt.bfloat16
F8 = mybir.dt.float8e4
AF = mybir.ActivationFunctionType
ALU = mybir.AluOpType
DR = mybir.MatmulPerfMode.DoubleRow

_CACHE = {}


def _r128(dram_ap):
    """[(m*128), f] DRAM view -> [128, m, f]"""
    return dram_ap.rearrange("(m p) f -> p m f", p=128)


def _build(phases='ABCD', gelu_sigmoid=False, debug_taps=False):
    # gelu_sigmoid: CoreSim-only fallback (interp lacks Gelu); approximates
    # gelu(z) as z*sigmoid(1.702z). The shipped kernel uses exact AF.Gelu.
    # debug_taps: add intermediate tensors as extra outputs (diagnostics only).
    nc = Bacc(trn_type='TRN2')

    # ---- DRAM I/O ----
    xT_d = nc.dram_tensor('xT', [C, T], BF16, kind='ExternalInput')
    xQ_d = nc.dram_tensor('xQ', [C, TQ], BF16, kind='ExternalInput')
    xqb_d = nc.dram_tensor('xqb', [C, TQ], F32, kind='ExternalInput')
    # Weights host-pretiled to [128, mt, kt, 128] fp8 (DoubleRow consumes kt pairs)
    wq_d = nc.dram_tensor('wq', [128, 8, 8, 128], F8, kind='ExternalInput')
    wk_d = nc.dram_tensor('wk', [128, 8, 8, 128], F8, kind='ExternalInput')
    wv_d = nc.dram_tensor('wv', [128, 8, C], F8, kind='ExternalInput')
    wproj_d = nc.dram_tensor('wproj', [128, 8, 8, 128], F8, kind='ExternalInput')
    wfc_d = nc.dram_tensor('wfc', [128, 32, 8, 128], F8, kind='ExternalInput')
    wfc2_d = nc.dram_tensor('wfc2', [128, 8, 32, 128], F8, kind='ExternalInput')
    bq_d = nc.dram_tensor('bq', [128, 8], F32, kind='ExternalInput')
    bk_d = nc.dram_tensor('bk', [128, 8], F32, kind='ExternalInput')
    bfc_d = nc.dram_tensor('bfc', [128, 32], F32, kind='ExternalInput')
    bfc2_d = nc.dram_tensor('bfc2', [128, 8], F32, kind='ExternalInput')
    alpha_d = nc.dram_tensor('alpha_b', [128, 1], F32, kind='ExternalInput')
    mask8_d = nc.dram_tensor('mask8', [128, 4, 128], F8, kind='ExternalInput')
    ones_d = nc.dram_tensor('ones_f8', [128, 16], F8, kind='ExternalInput')
    yT_d = nc.dram_tensor('yT', [C, TQ], F32, kind='ExternalOutput')
    taps = {}
    if debug_taps:
        for tn, shape, dt in [('tap_hT', [128, 8, T], F8),
                              ('tap_hQ', [128, 8, TQ], F8),
                              ('tap_Q', [128, 8, TQ], F8),
                              ('tap_K', [128, 8, T], F8),
                              ('tap_V', [128, 16, H, D + 1], F8),
                              ('tap_attnT', [128, 8, TQ], F8),
                              ('tap_x2', [128, 8, TQ], F32),
                              ('tap_gT', [128, 32, TQ], F8)]:
            taps[tn] = nc.dram_tensor(tn, shape, dt, kind='ExternalOutput')

    with TileContext(nc) as tc, ExitStack() as top:
        cpool = top.enter_context(tc.tile_pool(name='const', bufs=1))

        def cload(shape, dt, dram, tag):
            t = cpool.tile(shape, dt, tag=tag)
            nc.gpsimd.dma_start(t[:], dram[:])
            return t

        alpha_t = cload([128, 1], F32, alpha_d, 'c_alpha')
        bq_t = cload([128, 8], F32, bq_d, 'c_bq')
        bk_t = cload([128, 8], F32, bk_d, 'c_bk')
        bfc_t = cload([128, 32], F32, bfc_d, 'c_bfc')
        bfc2_t = cload([128, 8], F32, bfc2_d, 'c_bfc2')
        mask8_t = cload([128, 4, 128], F8, mask8_d, 'c_mask8')
        ones_t = cload([128, 16], F8, ones_d, 'c_ones')

        xT_r = _r128(xT_d[:])      # [128, 8, 2048]
        xQ_r = _r128(xQ_d[:])      # [128, 8, 512]
        xqb_r = _r128(xqb_d[:])    # [128, 8, 512]
        yT_r = _r128(yT_d[:])      # [128, 8, 512]

        # attnT outlives kqv (written in B, read in C); pools pop LIFO.
        attnT_pool = top.enter_context(tc.tile_pool(name='attnT', bufs=1))

        # Phase-C/D weights + residual, prefetched during phase B so the MLP
        # tail isn't DMA-bound. Pool opened before kqv (LIFO); DMAs emitted
        # between phases A and B.
        wpre = top.enter_context(tc.tile_pool(name='wpre', bufs=1))
        wproj_sb = wpre.tile([128, 8, 8, 128], F8)
        wfc_sb = wpre.tile([128, 32, 8, 128], F8)
        wfc2_sb = wpre.tile([128, 8, 32, 128], F8)
        xqb_t = wpre.tile([128, 8, TQ], F32)

        # K/Q/V buffers live through phases A+B
        es_kqv = ExitStack()
        kqv = es_kqv.enter_context(tc.tile_pool(name='kqv', bufs=1))
        K_f8 = kqv.tile([128, 8, T], F8)              # K^T (DR-permuted cols)
        Q_f8 = kqv.tile([128, 8, TQ], F8)             # Q^T (DR-permuted cols)
        V_f8 = kqv.tile([128, 16, H, D + 1], F8)      # token-major V + ones col

        # ====== Phases A+B fused: DyT + QKV projections + attention ======
        # Head emission interleaves with the K/V projection stream so the
        # Act engine's exp work starts while phase A's PE/DVE tail drains.
        with (
            tc.tile_pool(name='hT_pool', bufs=1) as hpool,
            tc.tile_pool(name='stageA', bufs=2) as spool,
            tc.tile_pool(name='wA', bufs=3) as wpool,
            tc.tile_pool(name='wvA', bufs=2) as wvpool,
            tc.tile_pool(name='pB', bufs=6) as pbpool,
            tc.tile_pool(name='pRec', bufs=2) as rpool,
            tc.tile_pool(name='psA', bufs=2, space='PSUM') as psA,
            tc.tile_pool(name='psS', bufs=2, space='PSUM') as psS,
            tc.tile_pool(name='psO', bufs=2, space='PSUM') as psO,
        ):
            hT = hpool.tile([128, 8, T], F8)
            hQ = hpool.tile([128, 8, TQ], F8)
            # hQ = DyT of this core's (strided) query tokens, host-gathered
            # into xQ so the SPMD program needs no per-core stride offsets.
            for k4 in range(2):
                xq = spool.tile([128, 4, TQ], BF16, tag='xstage')
                nc.sync.dma_start(xq[:], xQ_r[:, k4 * 4:(k4 + 1) * 4, :])
                nc.scalar.activation(hQ[:, k4 * 4:(k4 + 1) * 4, :],
                                     xq[:], AF.Tanh, scale=alpha_t[:, 0:1])
            # hT = tanh(alpha * x) quantized to fp8 (gamma/beta folded into
            # the consuming weights host-side). nt-outer so K-proj's first
            # group unblocks early.
            for nt in range(4):
                for k4 in range(2):
                    xt = spool.tile([128, 4, TQ], BF16, tag='xstage')
                    nc.sync.dma_start(
                        xt[:], xT_r[:, k4 * 4:(k4 + 1) * 4, nt * TQ:(nt + 1) * TQ])
                    nc.scalar.activation(
                        hT[:, k4 * 4:(k4 + 1) * 4, nt * TQ:(nt + 1) * TQ],
                        xt[:], AF.Tanh, scale=alpha_t[:, 0:1])

            # Q^T = wq^T @ hQ  (+bq), DoubleRow over kt pairs
            for mt in range(8):
                wt = wpool.tile([128, 8, 128], F8, tag='wkq')
                nc.sync.dma_start(wt[:], wq_d[:, mt])
                ps = psA.tile([128, TQ], F32)
                for kp in range(4):
                    nc.tensor.matmul(ps[:], wt[:, 2 * kp:2 * kp + 2, :],
                                     hQ[:, 2 * kp:2 * kp + 2, :],
                                     start=(kp == 0), stop=(kp == 3), perf_mode=DR)
                nc.vector.tensor_scalar(Q_f8[:, mt, :], ps[:],
                                        bq_t[:, mt:mt + 1], None, ALU.add)

            # K^T = wk^T @ hT (+bk, DVE evac) interleaved with
            # V = hT^T @ wv (token-major, Pool evac) so both evac engines
            # run concurrently. v-bias folded into xqb via w_proj.
            wk_tiles = []
            for mt in range(8):
                wt = wpool.tile([128, 8, 128], F8, tag=f'wkq{mt % 3}')
                nc.sync.dma_start(wt[:], wk_d[:, mt])
                wk_tiles.append(wt)
            wv_tiles = []
            for n2 in range(2):
                wvt = wvpool.tile([128, 8, TQ], F8, tag='wv')
                nc.sync.dma_start(wvt[:], wv_d[:, :, n2 * TQ:(n2 + 1) * TQ])
                wv_tiles.append(wvt)

            def k_part(i):
                mt, nt = i // 4, i % 4
                wt = wk_tiles[mt]
                ps = psA.tile([128, TQ], F32)
                for kp in range(4):
                    nc.tensor.matmul(ps[:], wt[:, 2 * kp:2 * kp + 2, :],
                                     hT[:, 2 * kp:2 * kp + 2, nt * TQ:(nt + 1) * TQ],
                                     start=(kp == 0), stop=(kp == 3), perf_mode=DR)
                nc.vector.tensor_scalar(K_f8[:, mt, nt * TQ:(nt + 1) * TQ],
                                        ps[:], bk_t[:, mt:mt + 1], None, ALU.add)

            def v_part(i):
                n2, kvb = i // 16, i % 16
                wvt = wv_tiles[n2]
                ps = psA.tile([128, TQ], F32)
                for kp in range(4):
                    nc.tensor.matmul(ps[:], hT[:, 2 * kp:2 * kp + 2, kvb * 128:(kvb + 1) * 128],
                                     wvt[:, 2 * kp:2 * kp + 2, :],
                                     start=(kp == 0), stop=(kp == 3), perf_mode=DR)
                # GPSIMD cannot read PSUM on hw; evac split: n2=0 half on Act
                # (dispatched mid-phase-A, before Act's in-order SEQ reaches
                # the exps), n2=1 half on DVE.
                if n2 == 0 or kvb % 2 == 1:
                    nc.scalar.activation(
                        V_f8[:, kvb, n2 * 8:(n2 + 1) * 8, 0:D],
                        ps[:].rearrange("p (h d) -> p h d", d=D), AF.Copy)
                else:
                    nc.vector.tensor_copy(
                        V_f8[:, kvb, n2 * 8:(n2 + 1) * 8, 0:D],
                        ps[:].rearrange("p (h d) -> p h d", d=D))

            # ones columns depend only on the const tile; emit before the
            # interleave so early heads' AV matmuls aren't blocked
            for kvb in range(16):
                nc.gpsimd.tensor_copy(V_f8[:, kvb, :, D], ones_t[:, :])

            attnT = attnT_pool.tile([128, 8, TQ], F8)

            # --- attention head body (strided-causal) ---
            # Query group k (cols [128k, 128k+128)) = strided tokens from the
            # original 512-token range k; kv quad q (blocks 4q..4q+3) is
            # needed only by groups k >= q, so quad q runs on query cols
            # [128q:512). Quad 0's first AV matmul covers the full 512
            # columns with start=True (zeroing the bank); later quads
            # accumulate into sub-ranges of already-written bytes
            # (skip_group_check since per-region stop can't be expressed).
            # Diagonal (group-q) columns get a post-exp 0/1 fp8 mask multiply.
            def head(h):
                t4, c4 = h // 4, h % 4
                hb = (h % 2) * 64
                hc = h // 2
                po = psO.tile([65, TQ], F32, tag='po')
                for q in (0, 1, 2, 3):
                    nq = (4 - q) * 128
                    for m2 in range(2):
                        ps = psS.tile([128, 2, TQ], F32, tag='score')
                        pt = pbpool.tile([128, 2, TQ], F8, tag='probs')
                        for j2 in range(2):
                            kvb = 4 * q + 2 * m2 + j2
                            nc.tensor.matmul(
                                ps[:, j2, 0:nq],
                                K_f8[32 * c4:32 * c4 + 32, 2 * t4:2 * t4 + 2, kvb * 128:(kvb + 1) * 128],
                                Q_f8[32 * c4:32 * c4 + 32, 2 * t4:2 * t4 + 2, 128 * q:TQ],
                                start=True, stop=True, perf_mode=DR,
                                tile_position=(32 * c4, 0))
                        nc.scalar.activation(pt[:, :, 0:nq], ps[:, :, 0:nq],
                                             AF.Exp, scale=1.0 / 8192.0)
                        meng = nc.vector if m2 == 0 else nc.gpsimd
                        meng.tensor_tensor(pt[:, :, 0:128], pt[:, :, 0:128],
                                           mask8_t[:, 2 * m2:2 * m2 + 2, :],
                                           ALU.mult)
                        nc.tensor.matmul(po[:, 128 * q:TQ],
                                         V_f8[:, 4 * q + 2 * m2:4 * q + 2 * m2 + 2, h, :],
                                         pt[:, :, 0:nq],
                                         start=(q == 0 and m2 == 0),
                                         stop=(q == 3 and m2 == 1), perf_mode=DR,
                                         skip_group_check=True)
                rec = rpool.tile([1, TQ], F32, tag='recip')
                nc.vector.reciprocal(rec[:], po[64:65, :])
                rec64 = rpool.tile([64, TQ], F32, tag='recip64')
                nc.gpsimd.partition_broadcast(rec64[:], rec[0:1, :])
                nc.vector.tensor_tensor(attnT[hb:hb + 64, hc, :], po[0:64, :],
                                        rec64[:], ALU.mult)

            def prefetch_cd():
                # Prefetch phase-C/D weights + residual during the attention
                # DMA-idle window. A tiny Pool write into each destination
                # (sourced from a mid-phase-A K evac) gives every DMA a WAR
                # dependency so the 9MB of prefetch traffic doesn't starve
                # phase A's own loads. Transfers issue on the idle SP queue.
                gate_src = K_f8[0:1, 4, 0:8]

                def gated_dma(dst_small, dst, src):
                    nc.gpsimd.tensor_copy(dst_small, gate_src)
                    nc.sync.dma_start(dst, src)

                gated_dma(xqb_t[0:1, 0, 0:8], xqb_t[:], xqb_r[:])
                gated_dma(wproj_sb[0:1, 0, 0, 0:8], wproj_sb[:], wproj_d[:])
                for mt4 in range(8):
                    gated_dma(wfc_sb[0:1, mt4 * 4, 0, 0:8],
                              wfc_sb[:, mt4 * 4:(mt4 + 1) * 4],
                              wfc_d[:, mt4 * 4:(mt4 + 1) * 4])
                for mt in range(8):
                    gated_dma(wfc2_sb[0:1, mt, 0, 0:8], wfc2_sb[:, mt],
                              wfc2_d[:, mt])

            # Interleave: after parts 0..15, K mt0-3 and the n2=0 V half are
            # done, which is everything heads 0..7 read besides late quads'
            # K columns (mt pairs are per head-group t4: heads 0-3 -> mt 0,1;
            # 4-7 -> mt 2,3). Early heads fill the Act trough at the A/B seam.
            heads = list(range(H if 'B' in phases else 0))

            def pop_heads(n):
                for _ in range(min(n, len(heads))):
                    head(heads.pop(0))

            # A head may only be emitted once every tile it reads has been
            # emitted (readers emitted before writers see stale memory):
            # heads 0-7 need K mt 0-3 (parts 0-15) and the full n2=0 V half
            # (parts 0-15); heads 8-15 need the n2=1 V half (parts 16-31).
            for i in range(16):
                k_part(i)
                v_part(i)
            pop_heads(4)
            for i in range(16, 20):
                k_part(i)
                v_part(i)
            pop_heads(2)
            for i in range(20, 24):
                k_part(i)
                v_part(i)
            prefetch_cd()
            pop_heads(2)
            for i in range(24, 32):
                k_part(i)
                v_part(i)
            pop_heads(len(heads))
            if debug_taps:
                nc.sync.dma_start(taps['tap_hT'][:], hT[:])
                nc.sync.dma_start(taps['tap_hQ'][:], hQ[:])
                nc.sync.dma_start(taps['tap_Q'][:], Q_f8[:])
                nc.sync.dma_start(taps['tap_K'][:], K_f8[:])
                nc.sync.dma_start(taps['tap_V'][:], V_f8[:])
        es_kqv.close()

        # x2T/h2T live through phases C+D
        es_mlp = ExitStack()
        mpool = es_mlp.enter_context(tc.tile_pool(name='mlp', bufs=1))
        x2T = mpool.tile([128, 8, TQ], F32)
        h2T = mpool.tile([128, 8, TQ], F8)

        # ======== Phases C+D in one scope ====
        with (
            tc.tile_pool(name='stageC', bufs=3) as scpool,
            tc.tile_pool(name='gT_pool', bufs=1) as gpool,
            tc.tile_pool(name='psC', bufs=4, space='PSUM') as psC,
        ):
            for mt in range(8 if 'C' in phases else 0):
                ps = psC.tile([128, TQ], F32)
                for kp in range(4):
                    nc.tensor.matmul(ps[:], wproj_sb[:, mt, 2 * kp:2 * kp + 2, :],
                                     attnT[:, 2 * kp:2 * kp + 2, :],
                                     start=(kp == 0), stop=(kp == 3), perf_mode=DR)
                # x2 = psum/1024 + (x + b_proj_eff)
                nc.vector.scalar_tensor_tensor(x2T[:, mt, :], ps[:], 1.0 / 1024.0,
                                               xqb_t[:, mt, :], ALU.mult, ALU.add)
                nc.scalar.activation(h2T[:, mt, :], x2T[:, mt, :], AF.Tanh,
                                     scale=alpha_t[:, 0:1])

            # ================= Phase D: MLP =================
            gT = gpool.tile([128, 32, TQ], F8)
            for mt in range(32 if 'D' in phases else 0):
                ps = psC.tile([128, TQ], F32)
                for kp in range(4):
                    nc.tensor.matmul(ps[:], wfc_sb[:, mt, 2 * kp:2 * kp + 2, :],
                                     h2T[:, 2 * kp:2 * kp + 2, :],
                                     start=(kp == 0), stop=(kp == 3), perf_mode=DR)
                if gelu_sigmoid:
                    zt = scpool.tile([128, TQ], F32, tag='gelu_z')
                    nc.vector.tensor_scalar(zt[:], ps[:], 1.0 / SW,
                                            bfc_t[:, mt:mt + 1], ALU.mult, ALU.add)
                    sg = scpool.tile([128, TQ], F32, tag='gelu_s')
                    nc.scalar.activation(sg[:], zt[:], AF.Sigmoid, scale=1.702)
                    nc.vector.tensor_tensor(gT[:, mt, :], zt[:], sg[:], ALU.mult)
                else:
                    nc.scalar.activation(gT[:, mt, :], ps[:], AF.Gelu,
                                         bias=bfc_t[:, mt:mt + 1], scale=1.0 / SW)

            # fc2 in two groups of 4 output tiles, kp-outer: each kp step
            # consumes gelu outputs as they land instead of serializing the
            # whole 16-step accumulation after the last gelu.
            if 'D' in phases:
                for g4 in range(2):
                    pss = [psC.tile([128, TQ], F32, tag=f'fc2_{m}', bufs=1,
                                    name=f'ps_fc2_{g4}_{m}')
                           for m in range(4)]
                    for kp in range(16):
                        for m in range(4):
                            nc.tensor.matmul(
                                pss[m][:],
                                wfc2_sb[:, g4 * 4 + m, 2 * kp:2 * kp + 2, :],
                                gT[:, 2 * kp:2 * kp + 2, :],
                                start=(kp == 0), stop=(kp == 15), perf_mode=DR)
                    for m in range(4):
                        mt = g4 * 4 + m
                        tmp = scpool.tile([128, TQ], F32, tag='bias2')
                        nc.vector.tensor_scalar(tmp[:], pss[m][:], 1.0 / SW,
                                                bfc2_t[:, mt:mt + 1], ALU.mult,
                                                ALU.add)
                        yt = scpool.tile([128, TQ], F32, tag='yout')
                        nc.vector.tensor_tensor(yt[:], tmp[:], x2T[:, mt, :],
                                                ALU.add)
                        nc.sync.dma_start(yT_r[:, mt, :], yt[:])
            if debug_taps:
                nc.sync.dma_start(taps['tap_attnT'][:], attnT[:])
                nc.sync.dma_start(taps['tap_x2'][:], x2T[:])
                nc.sync.dma_start(taps['tap_gT'][:], gT[:])
        es_mlp.close()

    nc.finalize()
    return nc


def _prep_inputs(x, alpha, gamma, beta, w_attn, b_attn, w_proj, b_proj,
                 w_fc, b_fc, w_fc2, b_fc2):
    f = np.float32
    f8 = ml_dtypes.float8_e4m3

    def tile_w(w, n_mt):
        # [K, M] -> [128, mt, kt, 128]: element [p, mt, kt, c] = w[kt*128+p, mt*128+c]
        kk, mm = w.shape
        return np.ascontiguousarray(
            w.reshape(kk // 128, 128, n_mt, 128).transpose(1, 2, 0, 3).astype(f8))

    # Fold DyT's gamma/beta into the consuming weights:
    #   w.T @ (g*t + b) = (g[:,None]*w).T @ t + (w.T @ b)
    g64 = np.asarray(gamma, np.float64)
    b64 = np.asarray(beta, np.float64)
    w64 = np.asarray(w_attn, np.float64)
    wp64 = np.asarray(w_proj, np.float64)
    wfc64 = np.asarray(w_fc, np.float64)
    wfc264 = np.asarray(w_fc2, np.float64)
    wq64, wk64, wv64 = w64[:, :C], w64[:, C:2 * C], w64[:, 2 * C:]
    bq_e = np.asarray(b_attn[:C], np.float64) + wq64.T @ b64
    bk_e = np.asarray(b_attn[C:2 * C], np.float64) + wk64.T @ b64
    bv_e = np.asarray(b_attn[2 * C:], np.float64) + wv64.T @ b64
    bfc_e = np.asarray(b_fc, np.float64) + wfc64.T @ b64
    # v-bias rides through attention (sum(p)=1) -> fold through w_proj
    bproj_e = np.asarray(b_proj, np.float64) + bv_e @ wp64

    # Column permutation for the scores-DoubleRow layout: m-tile mt=2t+i,
    # col c'=32c+r  <->  original col 64*(4t+c) + 32i + r  (head 4t+c, d=32i+r)
    mt_i = np.arange(8)
    cp = np.arange(128)
    tg = mt_i[:, None] // 2
    ig = mt_i[:, None] % 2
    cg = cp[None, :] // 32
    rg = cp[None, :] % 32
    perm = (256 * tg + 64 * cg + 32 * ig + rg).reshape(-1)

    wq_p = (SW * wq64 * g64[:, None])[:, perm]
    wk_p = (SW * wk64 * g64[:, None])[:, perm]
    bq_p = (SW * bq_e)[perm]
    bk_p = (SW * bk_e)[perm]

    wv = np.ascontiguousarray(
        (SW * wv64 * g64[:, None]).reshape(8, 128, C).transpose(1, 0, 2).astype(f8))
    bq = np.ascontiguousarray(bq_p.reshape(8, 128).T.astype(f))
    bk = np.ascontiguousarray(bk_p.reshape(8, 128).T.astype(f))
    bfc = np.ascontiguousarray(bfc_e.reshape(32, 128).T.astype(f))
    bfc2 = np.ascontiguousarray(
        np.asarray(b_fc2, np.float64).reshape(8, 128).T.astype(f))
    alpha_b = np.full((128, 1), float(np.asarray(alpha).reshape(-1)[0]), f)
    ones_f8 = np.ones((128, 16), f8)

    shared = dict(wq=tile_w(wq_p, 8), wk=tile_w(wk_p, 8), wv=wv,
                  wproj=tile_w(SW * wp64, 8),
                  wfc=tile_w(SW * wfc64 * g64[:, None], 32),
                  wfc2=tile_w(SW * wfc264, 8),
                  bq=bq, bk=bk, bfc=bfc, bfc2=bfc2,
                  alpha_b=alpha_b, ones_f8=ones_f8)

    in_maps = []
    for c in range(8):
        b, j = c // 4, c % 4
        xbT = np.asarray(x[b], f).T                       # [C, T] natural order
        xT = np.ascontiguousarray(xbT.astype(ml_dtypes.bfloat16))
        xQ = np.ascontiguousarray(xbT[:, j::4].astype(ml_dtypes.bfloat16))
        xqb = np.ascontiguousarray(
            (np.asarray(x[b, j::4, :], np.float64).T + bproj_e[:, None]).astype(f))
        # mask8[p, m, i] = keep iff key 128m+p <= query 4i+j (within a quad)
        r = np.arange(128)[:, None, None]
        mm = np.arange(4)[None, :, None]
        ii = np.arange(128)[None, None, :]
        mask8 = np.where(128 * mm + r <= 4 * ii + j, 1.0, 0.0).astype(f8)
        in_maps.append(dict(shared, xT=xT, xQ=xQ, xqb=xqb, mask8=mask8))
    return in_maps


def kernel(**inputs):
    if 'nc' not in _CACHE:
        _CACHE['nc'] = _build()
    nc = _CACHE['nc']
    in_maps = _prep_inputs(**inputs)
    res = run_bass_kernel_spmd(nc, in_maps, core_ids=list(range(8)))
    out = np.zeros((2, T, C), np.float32)
    for c in range(8):
        b, j = c // 4, c % 4
        out[b, j::4, :] = res.results[c]['yT'].T
    return out


# revision 59
# speedup vs baseline: 2.4618x; 1.0044x over previous
"""Trainium2 Bass kernel for a dense transformer block (DyT-norm causal attention + GELU MLP).

Sharding: 8 cores, SPMD single NEFF, no collectives. Core c handles batch b=c//4
and query tokens [qs*512:(qs+1)*512] with qs=c%4. Each core computes K/V for the
full sequence of its batch, attention for its query slice over all 16 heads, then
projection + MLP on its token slice. Outputs are disjoint; host gathers.

All large matmuls run as fp8e4m3 DoubleRow (2 contraction sub-tiles per
instruction, 0.5 PE cycles/row). Weights are host-scaled by 32 and quantized to
fp8; descales fold into PSUM-evacuation ops and activation scales. wq/wk columns
are host-permuted so Q^T/K^T land in SBUF as [32, 2, *] per head, letting the
64-deep score contraction also use DoubleRow. The attention@V matmul pairs two
whole kv blocks per DoubleRow instruction (equivalent to 2-step PSUM
accumulation). Softmax is unshifted exp with the denominator fused via a
ones-column on V; the V bias is folded through w_proj into the residual.

Causal masking: host permutes each core's key token order to [query-window |
earlier | later]. KV blocks 0-3 are the diagonal (additive -8e6 triangular mask
constants on the raw psum scores), remaining block-pairs use a per-core additive
bias column (0 / -30000) inside the softmax exp.
"""

import sys
from contextlib import ExitStack

for _p in ('/opt/trn_rl_repo',):
    if _p not in sys.path:
        sys.path.insert(0, _p)

import numpy as np
import ml_dtypes

import concourse.bass as bass
import concourse.mybir as mybir
from concourse.bacc import Bacc
from concourse.bass_utils import run_bass_kernel_spmd
from concourse.tile import TileContext

C = 1024
H = 16
D = 64
FF = 4096
T = 2048
TQ = 512          # query tokens per core
NEG = -30000.0    # exp bias-column mask (applied post-scale)
NEGM = -8.0e6     # additive score mask in raw psum units (pre 1/8192 scale)
SW = 32.0         # fp8 weight scale
F32 = mybir.dt.float32
BF16 = mybir.dt.bfloat16
F8 = mybir.dt.float8e4
AF = mybir.ActivationFunctionType
ALU = mybir.AluOpType
DR = mybir.MatmulPerfMode.DoubleRow

_CACHE = {}


def _r128(dram_ap):
    """[(m*128), f] DRAM view -> [128, m, f]"""
    return dram_ap.rearrange("(m p) f -> p m f", p=128)


def _build(phases='ABCD', gelu_sigmoid=False, debug_taps=False):
    # gelu_sigmoid: CoreSim-only fallback (interp lacks Gelu); approximates
    # gelu(z) as z*sigmoid(1.702z). The shipped kernel uses exact AF.Gelu.
    # debug_taps: add intermediate tensors as extra outputs (diagnostics only).
    nc = Bacc(trn_type='TRN2')

    # ---- DRAM I/O ----
    xT_d = nc.dram_tensor('xT', [C, T], BF16, kind='ExternalInput')
    xQ_d = nc.dram_tensor('xQ', [C, TQ], BF16, kind='ExternalInput')
    xqb_d = nc.dram_tensor('xqb', [C, TQ], F32, kind='ExternalInput')
    # Weights host-pretiled to [128, mt, kt, 128] fp8 (DoubleRow consumes kt pairs)
    wq_d = nc.dram_tensor('wq', [128, 8, 8, 128], F8, kind='ExternalInput')
    wk_d = nc.dram_tensor('wk', [128, 8, 8, 128], F8, kind='ExternalInput')
    wv_d = nc.dram_tensor('wv', [128, 8, C], F8, kind='ExternalInput')
    wproj_d = nc.dram_tensor('wproj', [128, 8, 8, 128], F8, kind='ExternalInput')
    wfc_d = nc.dram_tensor('wfc', [128, 32, 8, 128], F8, kind='ExternalInput')
    wfc2_d = nc.dram_tensor('wfc2', [128, 8, 32, 128], F8, kind='ExternalInput')
    bq_d = nc.dram_tensor('bq', [128, 8], F32, kind='ExternalInput')
    bk_d = nc.dram_tensor('bk', [128, 8], F32, kind='ExternalInput')
    bfc_d = nc.dram_tensor('bfc', [128, 32], F32, kind='ExternalInput')
    bfc2_d = nc.dram_tensor('bfc2', [128, 8], F32, kind='ExternalInput')
    alpha_d = nc.dram_tensor('alpha_b', [128, 1], F32, kind='ExternalInput')
    mask8_d = nc.dram_tensor('mask8', [128, 4, 128], F8, kind='ExternalInput')
    ones_d = nc.dram_tensor('ones_f8', [128, 16], F8, kind='ExternalInput')
    yT_d = nc.dram_tensor('yT', [C, TQ], F32, kind='ExternalOutput')
    taps = {}
    if debug_taps:
        for tn, shape, dt in [('tap_hT', [128, 8, T], F8),
                              ('tap_hQ', [128, 8, TQ], F8),
                              ('tap_Q', [128, 8, TQ], F8),
                              ('tap_K', [128, 8, T], F8),
                              ('tap_V', [128, 16, H, D + 1], F8),
                              ('tap_attnT', [128, 8, TQ], F8),
                              ('tap_x2', [128, 8, TQ], F32),
                              ('tap_gT', [128, 32, TQ], F8)]:
            taps[tn] = nc.dram_tensor(tn, shape, dt, kind='ExternalOutput')

    with TileContext(nc) as tc, ExitStack() as top:
        cpool = top.enter_context(tc.tile_pool(name='const', bufs=1))

        def cload(shape, dt, dram, tag):
            t = cpool.tile(shape, dt, tag=tag)
            nc.gpsimd.dma_start(t[:], dram[:])
            return t

        alpha_t = cload([128, 1], F32, alpha_d, 'c_alpha')
        bq_t = cload([128, 8], F32, bq_d, 'c_bq')
        bk_t = cload([128, 8], F32, bk_d, 'c_bk')
        bfc_t = cload([128, 32], F32, bfc_d, 'c_bfc')
        bfc2_t = cload([128, 8], F32, bfc2_d, 'c_bfc2')
        mask8_t = cload([128, 4, 128], F8, mask8_d, 'c_mask8')
        ones_t = cload([128, 16], F8, ones_d, 'c_ones')

        xT_r = _r128(xT_d[:])      # [128, 8, 2048]
        xQ_r = _r128(xQ_d[:])      # [128, 8, 512]
        xqb_r = _r128(xqb_d[:])    # [128, 8, 512]
        yT_r = _r128(yT_d[:])      # [128, 8, 512]

        # attnT outlives kqv (written in B, read in C); pools pop LIFO.
        attnT_pool = top.enter_context(tc.tile_pool(name='attnT', bufs=1))

        # Phase-C/D weights + residual, prefetched during phase B so the MLP
        # tail isn't DMA-bound. Pool opened before kqv (LIFO); DMAs emitted
        # between phases A and B.
        wpre = top.enter_context(tc.tile_pool(name='wpre', bufs=1))
        wproj_sb = wpre.tile([128, 8, 8, 128], F8)
        wfc_sb = wpre.tile([128, 32, 8, 128], F8)
        wfc2_sb = wpre.tile([128, 8, 32, 128], F8)
        xqb_t = wpre.tile([128, 8, TQ], F32)

        # K/Q/V buffers live through phases A+B
        es_kqv = ExitStack()
        kqv = es_kqv.enter_context(tc.tile_pool(name='kqv', bufs=1))
        K_f8 = kqv.tile([128, 8, T], F8)              # K^T (DR-permuted cols)
        Q_f8 = kqv.tile([128, 8, TQ], F8)             # Q^T (DR-permuted cols)
        V_f8 = kqv.tile([128, 16, H, D + 1], F8)      # token-major V + ones col
        pbpool = es_kqv.enter_context(tc.tile_pool(name='pB', bufs=6))
        rpool = es_kqv.enter_context(tc.tile_pool(name='pRec', bufs=2))

        # ====== Phases A+B fused: DyT + QKV projections + attention ======
        # Head emission interleaves with the K/V projection stream so the
        # Act engine's exp work starts while phase A's PE/DVE tail drains.
        with (
            tc.tile_pool(name='hT_pool', bufs=1) as hpool,
            tc.tile_pool(name='stageA', bufs=2) as spool,
            tc.tile_pool(name='wA', bufs=3) as wpool,
            tc.tile_pool(name='wvA', bufs=2) as wvpool,
            tc.tile_pool(name='psA', bufs=2, space='PSUM') as psA,
            tc.tile_pool(name='psS', bufs=2, space='PSUM') as psS,
            tc.tile_pool(name='psO', bufs=2, space='PSUM') as psO,
        ):
            hT = hpool.tile([128, 8, T], F8)
            hQ = hpool.tile([128, 8, TQ], F8)
            # hQ = DyT of this core's (strided) query tokens, host-gathered
            # into xQ so the SPMD program needs no per-core stride offsets.
            for k4 in range(2):
                xq = spool.tile([128, 4, TQ], BF16, tag='xstage')
                nc.sync.dma_start(xq[:], xQ_r[:, k4 * 4:(k4 + 1) * 4, :])
                nc.scalar.activation(hQ[:, k4 * 4:(k4 + 1) * 4, :],
                                     xq[:], AF.Tanh, scale=alpha_t[:, 0:1])
            # hT = tanh(alpha * x) quantized to fp8 (gamma/beta folded into
            # the consuming weights host-side). nt-outer so K-proj's first
            # group unblocks early.
            for nt in range(4):
                for k4 in range(2):
                    xt = spool.tile([128, 4, TQ], BF16, tag='xstage')
                    nc.sync.dma_start(
                        xt[:], xT_r[:, k4 * 4:(k4 + 1) * 4, nt * TQ:(nt + 1) * TQ])
                    nc.scalar.activation(
                        hT[:, k4 * 4:(k4 + 1) * 4, nt * TQ:(nt + 1) * TQ],
                        xt[:], AF.Tanh, scale=alpha_t[:, 0:1])

            # Q^T = wq^T @ hQ  (+bq), DoubleRow over kt pairs
            for mt in range(8):
                wt = wpool.tile([128, 8, 128], F8, tag='wkq')
                nc.sync.dma_start(wt[:], wq_d[:, mt])
                ps = psA.tile([128, TQ], F32)
                for kp in range(4):
                    nc.tensor.matmul(ps[:], wt[:, 2 * kp:2 * kp + 2, :],
                                     hQ[:, 2 * kp:2 * kp + 2, :],
                                     start=(kp == 0), stop=(kp == 3), perf_mode=DR)
                nc.vector.tensor_scalar(Q_f8[:, mt, :], ps[:],
                                        bq_t[:, mt:mt + 1], None, ALU.add)

            # K^T = wk^T @ hT (+bk, DVE evac) interleaved with
            # V = hT^T @ wv (token-major, Pool evac) so both evac engines
            # run concurrently. v-bias folded into xqb via w_proj.
            wk_tiles = []
            for mt in range(8):
                wt = wpool.tile([128, 8, 128], F8, tag=f'wkq{mt % 3}')
                nc.sync.dma_start(wt[:], wk_d[:, mt])
                wk_tiles.append(wt)
            wv_tiles = []
            for n2 in range(2):
                wvt = wvpool.tile([128, 8, TQ], F8, tag='wv')
                nc.sync.dma_start(wvt[:], wv_d[:, :, n2 * TQ:(n2 + 1) * TQ])
                wv_tiles.append(wvt)

            def k_part(i):
                mt, nt = i // 4, i % 4
                wt = wk_tiles[mt]
                ps = psA.tile([128, TQ], F32)
                for kp in range(4):
                    nc.tensor.matmul(ps[:], wt[:, 2 * kp:2 * kp + 2, :],
                                     hT[:, 2 * kp:2 * kp + 2, nt * TQ:(nt + 1) * TQ],
                                     start=(kp == 0), stop=(kp == 3), perf_mode=DR)
                nc.vector.tensor_scalar(K_f8[:, mt, nt * TQ:(nt + 1) * TQ],
                                        ps[:], bk_t[:, mt:mt + 1], None, ALU.add)

            def v_part(i):
                n2, kvb = i // 16, i % 16
                wvt = wv_tiles[n2]
                ps = psA.tile([128, TQ], F32)
                for kp in range(4):
                    nc.tensor.matmul(ps[:], hT[:, 2 * kp:2 * kp + 2, kvb * 128:(kvb + 1) * 128],
                                     wvt[:, 2 * kp:2 * kp + 2, :],
                                     start=(kp == 0), stop=(kp == 3), perf_mode=DR)
                # GPSIMD cannot read PSUM on hw; evac split: n2=0 half on Act
                # (dispatched mid-phase-A, before Act's in-order SEQ reaches
                # the exps), n2=1 half on DVE.
                if n2 == 0 or kvb % 2 == 1:
                    nc.scalar.activation(
                        V_f8[:, kvb, n2 * 8:(n2 + 1) * 8, 0:D],
                        ps[:].rearrange("p (h d) -> p h d", d=D), AF.Copy)
                else:
                    nc.vector.tensor_copy(
                        V_f8[:, kvb, n2 * 8:(n2 + 1) * 8, 0:D],
                        ps[:].rearrange("p (h d) -> p h d", d=D))

            # ones columns depend only on the const tile; emit before the
            # interleave so early heads' AV matmuls aren't blocked
            for kvb in range(16):
                nc.gpsimd.tensor_copy(V_f8[:, kvb, :, D], ones_t[:, :])

            attnT = attnT_pool.tile([128, 8, TQ], F8)

            # --- attention head body (strided-causal) ---
            # Query group k (cols [128k, 128k+128)) = strided tokens from the
            # original 512-token range k; kv quad q (blocks 4q..4q+3) is
            # needed only by groups k >= q, so quad q runs on query cols
            # [128q:512). Quad 0's first AV matmul covers the full 512
            # columns with start=True (zeroing the bank); later quads
            # accumulate into sub-ranges of already-written bytes
            # (skip_group_check since per-region stop can't be expressed).
            # Diagonal (group-q) columns get a post-exp 0/1 fp8 mask multiply.
            def head(h, psS=psS, psO=psO):
                t4, c4 = h // 4, h % 4
                hb = (h % 2) * 64
                hc = h // 2
                po = psO.tile([65, TQ], F32, tag='po')
                for q in (0, 1, 2, 3):
                    nq = (4 - q) * 128
                    for m2 in range(2):
                        ps = psS.tile([128, 2, TQ], F32, tag='score')
                        pt = pbpool.tile([128, 2, TQ], F8, tag='probs')
                        for j2 in range(2):
                            kvb = 4 * q + 2 * m2 + j2
                            nc.tensor.matmul(
                                ps[:, j2, 0:nq],
                                K_f8[32 * c4:32 * c4 + 32, 2 * t4:2 * t4 + 2, kvb * 128:(kvb + 1) * 128],
                                Q_f8[32 * c4:32 * c4 + 32, 2 * t4:2 * t4 + 2, 128 * q:TQ],
                                start=True, stop=True, perf_mode=DR,
                                tile_position=(32 * c4, 0))
                        nc.scalar.activation(pt[:, :, 0:nq], ps[:, :, 0:nq],
                                             AF.Exp, scale=1.0 / 8192.0)
                        meng = nc.vector if m2 == 0 else nc.gpsimd
                        meng.tensor_tensor(pt[:, :, 0:128], pt[:, :, 0:128],
                                           mask8_t[:, 2 * m2:2 * m2 + 2, :],
                                           ALU.mult)
                        nc.tensor.matmul(po[:, 128 * q:TQ],
                                         V_f8[:, 4 * q + 2 * m2:4 * q + 2 * m2 + 2, h, :],
                                         pt[:, :, 0:nq],
                                         start=(q == 0 and m2 == 0),
                                         stop=(q == 3 and m2 == 1), perf_mode=DR,
                                         skip_group_check=True)
                rec = rpool.tile([1, TQ], F32, tag='recip')
                nc.vector.reciprocal(rec[:], po[64:65, :])
                rec64 = rpool.tile([64, TQ], F32, tag='recip64')
                nc.gpsimd.partition_broadcast(rec64[:], rec[0:1, :])
                nc.vector.tensor_tensor(attnT[hb:hb + 64, hc, :], po[0:64, :],
                                        rec64[:], ALU.mult)

            def prefetch_cd():
                # Prefetch phase-C/D weights + residual during the attention
                # DMA-idle window. A tiny Pool write into each destination
                # (sourced from a mid-phase-A K evac) gives every DMA a WAR
                # dependency so the 9MB of prefetch traffic doesn't starve
                # phase A's own loads. Transfers issue on the idle SP queue.
                gate_src = K_f8[0:1, 4, 0:8]

                def gated_dma(dst_small, dst, src):
                    nc.gpsimd.tensor_copy(dst_small, gate_src)
                    nc.sync.dma_start(dst, src)

                gated_dma(xqb_t[0:1, 0, 0:8], xqb_t[:], xqb_r[:])
                gated_dma(wproj_sb[0:1, 0, 0, 0:8], wproj_sb[:], wproj_d[:])
                for mt4 in range(8):
                    gated_dma(wfc_sb[0:1, mt4 * 4, 0, 0:8],
                              wfc_sb[:, mt4 * 4:(mt4 + 1) * 4],
                              wfc_d[:, mt4 * 4:(mt4 + 1) * 4])
                for mt in range(8):
                    gated_dma(wfc2_sb[0:1, mt, 0, 0:8], wfc2_sb[:, mt],
                              wfc2_d[:, mt])

            # Interleave: after parts 0..15, K mt0-3 and the n2=0 V half are
            # done, which is everything heads 0..7 read besides late quads'
            # K columns (mt pairs are per head-group t4: heads 0-3 -> mt 0,1;
            # 4-7 -> mt 2,3). Early heads fill the Act trough at the A/B seam.
            heads = list(range(H if 'B' in phases else 0))

            def pop_heads(n, **kw):
                for _ in range(min(n, len(heads))):
                    head(heads.pop(0), **kw)

            # A head may only be emitted once every tile it reads has been
            # emitted (readers emitted before writers see stale memory):
            # heads 0-7 need K mt 0-3 (parts 0-15) and the full n2=0 V half
            # (parts 0-15); heads 8-15 need the n2=1 V half (parts 16-31).
            for i in range(16):
                k_part(i)
                v_part(i)
            pop_heads(4)
            for i in range(16, 20):
                k_part(i)
                v_part(i)
            pop_heads(2)
            for i in range(20, 24):
                k_part(i)
                v_part(i)
            prefetch_cd()
            pop_heads(2)
            for i in range(24, 32):
                k_part(i)
                v_part(i)
            if debug_taps:
                nc.sync.dma_start(taps['tap_hT'][:], hT[:])
                nc.sync.dma_start(taps['tap_hQ'][:], hQ[:])
        # Late heads get a deeper score-buffer rotation (3 tiles / 6 banks)
        # in the banks phase A's psA pool just released, keeping exp fed.
        with (
            tc.tile_pool(name='psS2', bufs=3, space='PSUM') as psS2,
            tc.tile_pool(name='psO2', bufs=2, space='PSUM') as psO2,
        ):
            pop_heads(len(heads), psS=psS2, psO=psO2)
            if debug_taps:
                nc.sync.dma_start(taps['tap_Q'][:], Q_f8[:])
                nc.sync.dma_start(taps['tap_K'][:], K_f8[:])
                nc.sync.dma_start(taps['tap_V'][:], V_f8[:])
        es_kqv.close()

        # x2T/h2T live through phases C+D
        es_mlp = ExitStack()
        mpool = es_mlp.enter_context(tc.tile_pool(name='mlp', bufs=1))
        x2T = mpool.tile([128, 8, TQ], F32)
        h2T = mpool.tile([128, 8, TQ], F8)

        # ======== Phases C+D in one scope ====
        with (
            tc.tile_pool(name='stageC', bufs=3) as scpool,
            tc.tile_pool(name='gT_pool', bufs=1) as gpool,
            tc.tile_pool(name='psC', bufs=4, space='PSUM') as psC,
        ):
            for mt in range(8 if 'C' in phases else 0):
                ps = psC.tile([128, TQ], F32)
                for kp in range(4):
                    nc.tensor.matmul(ps[:], wproj_sb[:, mt, 2 * kp:2 * kp + 2, :],
                                     attnT[:, 2 * kp:2 * kp + 2, :],
                                     start=(kp == 0), stop=(kp == 3), perf_mode=DR)
                # x2 = psum/1024 + (x + b_proj_eff)
                nc.vector.scalar_tensor_tensor(x2T[:, mt, :], ps[:], 1.0 / 1024.0,
                                               xqb_t[:, mt, :], ALU.mult, ALU.add)
                nc.scalar.activation(h2T[:, mt, :], x2T[:, mt, :], AF.Tanh,
                                     scale=alpha_t[:, 0:1])

            # ================= Phase D: MLP =================
            gT = gpool.tile([128, 32, TQ], F8)
            for mt in range(32 if 'D' in phases else 0):
                ps = psC.tile([128, TQ], F32)
                for kp in range(4):
                    nc.tensor.matmul(ps[:], wfc_sb[:, mt, 2 * kp:2 * kp + 2, :],
                                     h2T[:, 2 * kp:2 * kp + 2, :],
                                     start=(kp == 0), stop=(kp == 3), perf_mode=DR)
                if gelu_sigmoid:
                    zt = scpool.tile([128, TQ], F32, tag='gelu_z')
                    nc.vector.tensor_scalar(zt[:], ps[:], 1.0 / SW,
                                            bfc_t[:, mt:mt + 1], ALU.mult, ALU.add)
                    sg = scpool.tile([128, TQ], F32, tag='gelu_s')
                    nc.scalar.activation(sg[:], zt[:], AF.Sigmoid, scale=1.702)
                    nc.vector.tensor_tensor(gT[:, mt, :], zt[:], sg[:], ALU.mult)
                else:
                    nc.scalar.activation(gT[:, mt, :], ps[:], AF.Gelu,
                                         bias=bfc_t[:, mt:mt + 1], scale=1.0 / SW)

            # fc2 in two groups of 4 output tiles, kp-outer: each kp step
            # consumes gelu outputs as they land instead of serializing the
            # whole 16-step accumulation after the last gelu.
            if 'D' in phases:
                for g4 in range(2):
                    pss = [psC.tile([128, TQ], F32, tag=f'fc2_{m}', bufs=1,
                                    name=f'ps_fc2_{g4}_{m}')
                           for m in range(4)]
                    for kp in range(16):
                        for m in range(4):
                            nc.tensor.matmul(
                                pss[m][:],
                                wfc2_sb[:, g4 * 4 + m, 2 * kp:2 * kp + 2, :],
                                gT[:, 2 * kp:2 * kp + 2, :],
                                start=(kp == 0), stop=(kp == 15), perf_mode=DR)
                    for m in range(4):
                        mt = g4 * 4 + m
                        tmp = scpool.tile([128, TQ], F32, tag='bias2')
                        nc.vector.tensor_scalar(tmp[:], pss[m][:], 1.0 / SW,
                                                bfc2_t[:, mt:mt + 1], ALU.mult,
                                                ALU.add)
                        yt = scpool.tile([128, TQ], F32, tag='yout')
                        nc.vector.tensor_tensor(yt[:], tmp[:], x2T[:, mt, :],
                                                ALU.add)
                        nc.sync.dma_start(yT_r[:, mt, :], yt[:])
            if debug_taps:
                nc.sync.dma_start(taps['tap_attnT'][:], attnT[:])
                nc.sync.dma_start(taps['tap_x2'][:], x2T[:])
                nc.sync.dma_start(taps['tap_gT'][:], gT[:])
        es_mlp.close()

    nc.finalize()
    return nc


def _prep_inputs(x, alpha, gamma, beta, w_attn, b_attn, w_proj, b_proj,
                 w_fc, b_fc, w_fc2, b_fc2):
    f = np.float32
    f8 = ml_dtypes.float8_e4m3

    def tile_w(w, n_mt):
        # [K, M] -> [128, mt, kt, 128]: element [p, mt, kt, c] = w[kt*128+p, mt*128+c]
        kk, mm = w.shape
        return np.ascontiguousarray(
            w.reshape(kk // 128, 128, n_mt, 128).transpose(1, 2, 0, 3).astype(f8))

    # Fold DyT's gamma/beta into the consuming weights:
    #   w.T @ (g*t + b) = (g[:,None]*w).T @ t + (w.T @ b)
    g64 = np.asarray(gamma, np.float64)
    b64 = np.asarray(beta, np.float64)
    w64 = np.asarray(w_attn, np.float64)
    wp64 = np.asarray(w_proj, np.float64)
    wfc64 = np.asarray(w_fc, np.float64)
    wfc264 = np.asarray(w_fc2, np.float64)
    wq64, wk64, wv64 = w64[:, :C], w64[:, C:2 * C], w64[:, 2 * C:]
    bq_e = np.asarray(b_attn[:C], np.float64) + wq64.T @ b64
    bk_e = np.asarray(b_attn[C:2 * C], np.float64) + wk64.T @ b64
    bv_e = np.asarray(b_attn[2 * C:], np.float64) + wv64.T @ b64
    bfc_e = np.asarray(b_fc, np.float64) + wfc64.T @ b64
    # v-bias rides through attention (sum(p)=1) -> fold through w_proj
    bproj_e = np.asarray(b_proj, np.float64) + bv_e @ wp64

    # Column permutation for the scores-DoubleRow layout: m-tile mt=2t+i,
    # col c'=32c+r  <->  original col 64*(4t+c) + 32i + r  (head 4t+c, d=32i+r)
    mt_i = np.arange(8)
    cp = np.arange(128)
    tg = mt_i[:, None] // 2
    ig = mt_i[:, None] % 2
    cg = cp[None, :] // 32
    rg = cp[None, :] % 32
    perm = (256 * tg + 64 * cg + 32 * ig + rg).reshape(-1)

    wq_p = (SW * wq64 * g64[:, None])[:, perm]
    wk_p = (SW * wk64 * g64[:, None])[:, perm]
    bq_p = (SW * bq_e)[perm]
    bk_p = (SW * bk_e)[perm]

    wv = np.ascontiguousarray(
        (SW * wv64 * g64[:, None]).reshape(8, 128, C).transpose(1, 0, 2).astype(f8))
    bq = np.ascontiguousarray(bq_p.reshape(8, 128).T.astype(f))
    bk = np.ascontiguousarray(bk_p.reshape(8, 128).T.astype(f))
    bfc = np.ascontiguousarray(bfc_e.reshape(32, 128).T.astype(f))
    bfc2 = np.ascontiguousarray(
        np.asarray(b_fc2, np.float64).reshape(8, 128).T.astype(f))
    alpha_b = np.full((128, 1), float(np.asarray(alpha).reshape(-1)[0]), f)
    ones_f8 = np.ones((128, 16), f8)

    shared = dict(wq=tile_w(wq_p, 8), wk=tile_w(wk_p, 8), wv=wv,
                  wproj=tile_w(SW * wp64, 8),
                  wfc=tile_w(SW * wfc64 * g64[:, None], 32),
                  wfc2=tile_w(SW * wfc264, 8),
                  bq=bq, bk=bk, bfc=bfc, bfc2=bfc2,
                  alpha_b=alpha_b, ones_f8=ones_f8)

    in_maps = []
    for c in range(8):
        b, j = c // 4, c % 4
        xbT = np.asarray(x[b], f).T                       # [C, T] natural order
        xT = np.ascontiguousarray(xbT.astype(ml_dtypes.bfloat16))
        xQ = np.ascontiguousarray(xbT[:, j::4].astype(ml_dtypes.bfloat16))
        xqb = np.ascontiguousarray(
            (np.asarray(x[b, j::4, :], np.float64).T + bproj_e[:, None]).astype(f))
        # mask8[p, m, i] = keep iff key 128m+p <= query 4i+j (within a quad)
        r = np.arange(128)[:, None, None]
        mm = np.arange(4)[None, :, None]
        ii = np.arange(128)[None, None, :]
        mask8 = np.where(128 * mm + r <= 4 * ii + j, 1.0, 0.0).astype(f8)
        in_maps.append(dict(shared, xT=xT, xQ=xQ, xqb=xqb, mask8=mask8))
    return in_maps


def kernel(**inputs):
    if 'nc' not in _CACHE:
        _CACHE['nc'] = _build()
    nc = _CACHE['nc']
    in_maps = _prep_inputs(**inputs)
    res = run_bass_kernel_spmd(nc, in_maps, core_ids=list(range(8)))
    out = np.zeros((2, T, C), np.float32)
    for c in range(8):
        b, j = c // 4, c % 4
        out[b, j::4, :] = res.results[c]['yT'].T
    return out


# revision 60
# speedup vs baseline: 2.4855x; 1.0096x over previous
"""Trainium2 Bass kernel for a dense transformer block (DyT-norm causal attention + GELU MLP).

Sharding: 8 cores, SPMD single NEFF, no collectives. Core c handles batch b=c//4
and query tokens [qs*512:(qs+1)*512] with qs=c%4. Each core computes K/V for the
full sequence of its batch, attention for its query slice over all 16 heads, then
projection + MLP on its token slice. Outputs are disjoint; host gathers.

All large matmuls run as fp8e4m3 DoubleRow (2 contraction sub-tiles per
instruction, 0.5 PE cycles/row). Weights are host-scaled by 32 and quantized to
fp8; descales fold into PSUM-evacuation ops and activation scales. wq/wk columns
are host-permuted so Q^T/K^T land in SBUF as [32, 2, *] per head, letting the
64-deep score contraction also use DoubleRow. The attention@V matmul pairs two
whole kv blocks per DoubleRow instruction (equivalent to 2-step PSUM
accumulation). Softmax is unshifted exp with the denominator fused via a
ones-column on V; the V bias is folded through w_proj into the residual.

Causal masking: host permutes each core's key token order to [query-window |
earlier | later]. KV blocks 0-3 are the diagonal (additive -8e6 triangular mask
constants on the raw psum scores), remaining block-pairs use a per-core additive
bias column (0 / -30000) inside the softmax exp.
"""

import sys
from contextlib import ExitStack

for _p in ('/opt/trn_rl_repo',):
    if _p not in sys.path:
        sys.path.insert(0, _p)

import numpy as np
import ml_dtypes

import concourse.bass as bass
import concourse.mybir as mybir
from concourse.bacc import Bacc
from concourse.bass_utils import run_bass_kernel_spmd
from concourse.tile import TileContext

C = 1024
H = 16
D = 64
FF = 4096
T = 2048
TQ = 512          # query tokens per core
NEG = -30000.0    # exp bias-column mask (applied post-scale)
NEGM = -8.0e6     # additive score mask in raw psum units (pre 1/8192 scale)
SW = 32.0         # fp8 weight scale
F32 = mybir.dt.float32
BF16 = mybir.dt.bfloat16
F8 = mybir.dt.float8e4
AF = mybir.ActivationFunctionType
ALU = mybir.AluOpType
DR = mybir.MatmulPerfMode.DoubleRow

_CACHE = {}


def _r128(dram_ap):
    """[(m*128), f] DRAM view -> [128, m, f]"""
    return dram_ap.rearrange("(m p) f -> p m f", p=128)


def _build(phases='ABCD', gelu_sigmoid=False, debug_taps=False):
    # gelu_sigmoid: CoreSim-only fallback (interp lacks Gelu); approximates
    # gelu(z) as z*sigmoid(1.702z). The shipped kernel uses exact AF.Gelu.
    # debug_taps: add intermediate tensors as extra outputs (diagnostics only).
    nc = Bacc(trn_type='TRN2')

    # ---- DRAM I/O ----
    xT_d = nc.dram_tensor('xT', [C, T], BF16, kind='ExternalInput')
    xQ_d = nc.dram_tensor('xQ', [C, TQ], BF16, kind='ExternalInput')
    xqb_d = nc.dram_tensor('xqb', [C, TQ], F32, kind='ExternalInput')
    # Weights host-pretiled to [128, mt, kt, 128] fp8 (DoubleRow consumes kt pairs)
    wq_d = nc.dram_tensor('wq', [128, 8, 8, 128], F8, kind='ExternalInput')
    wk_d = nc.dram_tensor('wk', [128, 8, 8, 128], F8, kind='ExternalInput')
    wv_d = nc.dram_tensor('wv', [128, 8, C], F8, kind='ExternalInput')
    wproj_d = nc.dram_tensor('wproj', [128, 8, 8, 128], F8, kind='ExternalInput')
    wfc_d = nc.dram_tensor('wfc', [128, 32, 8, 128], F8, kind='ExternalInput')
    wfc2_d = nc.dram_tensor('wfc2', [128, 8, 32, 128], F8, kind='ExternalInput')
    bq_d = nc.dram_tensor('bq', [128, 8], F32, kind='ExternalInput')
    bk_d = nc.dram_tensor('bk', [128, 8], F32, kind='ExternalInput')
    bfc_d = nc.dram_tensor('bfc', [128, 32], F32, kind='ExternalInput')
    bfc2_d = nc.dram_tensor('bfc2', [128, 8], F32, kind='ExternalInput')
    alpha_d = nc.dram_tensor('alpha_b', [128, 1], F32, kind='ExternalInput')
    mask8_d = nc.dram_tensor('mask8', [128, 4, 128], F8, kind='ExternalInput')
    ones_d = nc.dram_tensor('ones_f8', [128, 16], F8, kind='ExternalInput')
    yT_d = nc.dram_tensor('yT', [C, TQ], F32, kind='ExternalOutput')
    taps = {}
    if debug_taps:
        for tn, shape, dt in [('tap_hT', [128, 8, T], F8),
                              ('tap_hQ', [128, 8, TQ], F8),
                              ('tap_Q', [128, 8, TQ], F8),
                              ('tap_K', [128, 8, T], F8),
                              ('tap_V', [128, 16, H, D + 1], F8),
                              ('tap_attnT', [128, 8, TQ], F8),
                              ('tap_x2', [128, 8, TQ], F32),
                              ('tap_gT', [128, 32, TQ], F8)]:
            taps[tn] = nc.dram_tensor(tn, shape, dt, kind='ExternalOutput')

    with TileContext(nc) as tc, ExitStack() as top:
        cpool = top.enter_context(tc.tile_pool(name='const', bufs=1))

        def cload(shape, dt, dram, tag):
            t = cpool.tile(shape, dt, tag=tag)
            nc.gpsimd.dma_start(t[:], dram[:])
            return t

        alpha_t = cload([128, 1], F32, alpha_d, 'c_alpha')
        bq_t = cload([128, 8], F32, bq_d, 'c_bq')
        bk_t = cload([128, 8], F32, bk_d, 'c_bk')
        bfc_t = cload([128, 32], F32, bfc_d, 'c_bfc')
        bfc2_t = cload([128, 8], F32, bfc2_d, 'c_bfc2')
        mask8_t = cload([128, 4, 128], F8, mask8_d, 'c_mask8')
        ones_t = cload([128, 16], F8, ones_d, 'c_ones')

        xT_r = _r128(xT_d[:])      # [128, 8, 2048]
        xQ_r = _r128(xQ_d[:])      # [128, 8, 512]
        xqb_r = _r128(xqb_d[:])    # [128, 8, 512]
        yT_r = _r128(yT_d[:])      # [128, 8, 512]

        # attnT outlives kqv (written in B, read in C); pools pop LIFO.
        attnT_pool = top.enter_context(tc.tile_pool(name='attnT', bufs=1))

        # Phase-C/D weights + residual, prefetched during phase B so the MLP
        # tail isn't DMA-bound. Pool opened before kqv (LIFO); DMAs emitted
        # between phases A and B.
        wpre = top.enter_context(tc.tile_pool(name='wpre', bufs=1))
        wproj_sb = wpre.tile([128, 8, 8, 128], F8)
        wfc_sb = wpre.tile([128, 32, 8, 128], F8)
        wfc2_sb = wpre.tile([128, 8, 32, 128], F8)
        xqb_t = wpre.tile([128, 8, TQ], F32)

        # K/Q/V buffers live through phases A+B
        es_kqv = ExitStack()
        kqv = es_kqv.enter_context(tc.tile_pool(name='kqv', bufs=1))
        K_f8 = kqv.tile([128, 8, T], F8)              # K^T (DR-permuted cols)
        Q_f8 = kqv.tile([128, 8, TQ], F8)             # Q^T (DR-permuted cols)
        V_f8 = kqv.tile([128, 16, H, D + 1], F8)      # token-major V + ones col
        pbpool = es_kqv.enter_context(tc.tile_pool(name='pB', bufs=6))
        rpool = es_kqv.enter_context(tc.tile_pool(name='pRec', bufs=2))

        # ====== Phases A+B fused: DyT + QKV projections + attention ======
        # Head emission interleaves with the K/V projection stream so the
        # Act engine's exp work starts while phase A's PE/DVE tail drains.
        with (
            tc.tile_pool(name='hT_pool', bufs=1) as hpool,
            tc.tile_pool(name='stageA', bufs=2) as spool,
            tc.tile_pool(name='wA', bufs=3) as wpool,
            tc.tile_pool(name='wvA', bufs=2) as wvpool,
            tc.tile_pool(name='psA', bufs=2, space='PSUM') as psA,
            tc.tile_pool(name='psS', bufs=2, space='PSUM') as psS,
            tc.tile_pool(name='psO', bufs=2, space='PSUM') as psO,
        ):
            hT = hpool.tile([128, 8, T], F8)
            hQ = hpool.tile([128, 8, TQ], F8)
            # hQ = DyT of this core's (strided) query tokens, host-gathered
            # into xQ so the SPMD program needs no per-core stride offsets.
            for k4 in range(2):
                xq = spool.tile([128, 4, TQ], BF16, tag='xstage')
                nc.sync.dma_start(xq[:], xQ_r[:, k4 * 4:(k4 + 1) * 4, :])
                nc.scalar.activation(hQ[:, k4 * 4:(k4 + 1) * 4, :],
                                     xq[:], AF.Tanh, scale=alpha_t[:, 0:1])
            # hT = tanh(alpha * x) quantized to fp8 (gamma/beta folded into
            # the consuming weights host-side). nt-outer so K-proj's first
            # group unblocks early.
            for nt in range(4):
                for k4 in range(2):
                    xt = spool.tile([128, 4, TQ], BF16, tag='xstage')
                    nc.sync.dma_start(
                        xt[:], xT_r[:, k4 * 4:(k4 + 1) * 4, nt * TQ:(nt + 1) * TQ])
                    nc.scalar.activation(
                        hT[:, k4 * 4:(k4 + 1) * 4, nt * TQ:(nt + 1) * TQ],
                        xt[:], AF.Tanh, scale=alpha_t[:, 0:1])

            # Q^T = wq^T @ hQ  (+bq), DoubleRow over kt pairs
            for mt in range(8):
                wt = wpool.tile([128, 8, 128], F8, tag='wkq')
                nc.sync.dma_start(wt[:], wq_d[:, mt])
                ps = psA.tile([128, TQ], F32)
                for kp in range(4):
                    nc.tensor.matmul(ps[:], wt[:, 2 * kp:2 * kp + 2, :],
                                     hQ[:, 2 * kp:2 * kp + 2, :],
                                     start=(kp == 0), stop=(kp == 3), perf_mode=DR)
                nc.vector.tensor_scalar(Q_f8[:, mt, :], ps[:],
                                        bq_t[:, mt:mt + 1], None, ALU.add)

            # K^T = wk^T @ hT (+bk, DVE evac) interleaved with
            # V = hT^T @ wv (token-major, Pool evac) so both evac engines
            # run concurrently. v-bias folded into xqb via w_proj.
            wk_tiles = []
            for mt in range(8):
                wt = wpool.tile([128, 8, 128], F8, tag=f'wkq{mt % 3}')
                nc.sync.dma_start(wt[:], wk_d[:, mt])
                wk_tiles.append(wt)
            wv_tiles = []
            for n2 in range(2):
                wvt = wvpool.tile([128, 8, TQ], F8, tag='wv')
                nc.sync.dma_start(wvt[:], wv_d[:, :, n2 * TQ:(n2 + 1) * TQ])
                wv_tiles.append(wvt)

            def k_part(i):
                mt, nt = i // 4, i % 4
                wt = wk_tiles[mt]
                ps = psA.tile([128, TQ], F32)
                for kp in range(4):
                    nc.tensor.matmul(ps[:], wt[:, 2 * kp:2 * kp + 2, :],
                                     hT[:, 2 * kp:2 * kp + 2, nt * TQ:(nt + 1) * TQ],
                                     start=(kp == 0), stop=(kp == 3), perf_mode=DR)
                nc.vector.tensor_scalar(K_f8[:, mt, nt * TQ:(nt + 1) * TQ],
                                        ps[:], bk_t[:, mt:mt + 1], None, ALU.add)

            def v_part(i):
                n2, kvb = i // 16, i % 16
                wvt = wv_tiles[n2]
                ps = psA.tile([128, TQ], F32)
                for kp in range(4):
                    nc.tensor.matmul(ps[:], hT[:, 2 * kp:2 * kp + 2, kvb * 128:(kvb + 1) * 128],
                                     wvt[:, 2 * kp:2 * kp + 2, :],
                                     start=(kp == 0), stop=(kp == 3), perf_mode=DR)
                # GPSIMD cannot read PSUM on hw; evac split: n2=0 half on Act
                # (dispatched mid-phase-A, before Act's in-order SEQ reaches
                # the exps), n2=1 half on DVE.
                if n2 == 0 or kvb % 2 == 1:
                    nc.scalar.activation(
                        V_f8[:, kvb, n2 * 8:(n2 + 1) * 8, 0:D],
                        ps[:].rearrange("p (h d) -> p h d", d=D), AF.Copy)
                else:
                    nc.vector.tensor_copy(
                        V_f8[:, kvb, n2 * 8:(n2 + 1) * 8, 0:D],
                        ps[:].rearrange("p (h d) -> p h d", d=D))

            # ones columns depend only on the const tile; emit before the
            # interleave so early heads' AV matmuls aren't blocked
            for kvb in range(16):
                nc.gpsimd.tensor_copy(V_f8[:, kvb, :, D], ones_t[:, :])

            attnT = attnT_pool.tile([128, 8, TQ], F8)

            # --- attention head body (strided-causal) ---
            # Query group k (cols [128k, 128k+128)) = strided tokens from the
            # original 512-token range k; kv quad q (blocks 4q..4q+3) is
            # needed only by groups k >= q, so quad q runs on query cols
            # [128q:512). Quad 0's first AV matmul covers the full 512
            # columns with start=True (zeroing the bank); later quads
            # accumulate into sub-ranges of already-written bytes
            # (skip_group_check since per-region stop can't be expressed).
            # Diagonal (group-q) columns get a post-exp 0/1 fp8 mask multiply.
            def head(h, psS=psS, psO=psO):
                t4, c4 = h // 4, h % 4
                hb = (h % 2) * 64
                hc = h // 2
                kq = dict(perf_mode=DR, tile_position=(32 * c4, 0))

                def kf(kvb):
                    return K_f8[32 * c4:32 * c4 + 32, 2 * t4:2 * t4 + 2,
                                kvb * 128:(kvb + 1) * 128]

                def qf(q):
                    return Q_f8[32 * c4:32 * c4 + 32, 2 * t4:2 * t4 + 2,
                                128 * q:TQ]

                po = psO.tile([65, TQ], F32, tag='po')
                for q in (0, 1):
                    nq = (4 - q) * 128
                    for m2 in range(2):
                        ps = psS.tile([128, 2, TQ], F32, tag='score')
                        pt = pbpool.tile([128, 2, TQ], F8, tag='probs')
                        for j2 in range(2):
                            nc.tensor.matmul(ps[:, j2, 0:nq],
                                             kf(4 * q + 2 * m2 + j2), qf(q),
                                             start=True, stop=True, **kq)
                        nc.scalar.activation(pt[:, :, 0:nq], ps[:, :, 0:nq],
                                             AF.Exp, scale=1.0 / 8192.0)
                        meng = nc.vector if m2 == 0 else nc.gpsimd
                        meng.tensor_tensor(pt[:, :, 0:128], pt[:, :, 0:128],
                                           mask8_t[:, 2 * m2:2 * m2 + 2, :],
                                           ALU.mult)
                        nc.tensor.matmul(po[:, 128 * q:TQ],
                                         V_f8[:, 4 * q + 2 * m2:4 * q + 2 * m2 + 2, h, :],
                                         pt[:, :, 0:nq],
                                         start=(q == 0 and m2 == 0),
                                         stop=False, perf_mode=DR,
                                         skip_group_check=True)
                # Quads 2 and 3 pack all four blocks contiguously in one tile
                # so exp and the diagonal mask are single ops (two score
                # blocks share a bank -> skip_group_check on the scores too).
                ps = psS.tile([128, 2, TQ], F32, tag='score')
                pt = pbpool.tile([128, 2, TQ], F8, tag='probs')
                for mm in range(4):   # quad 2: block mm at [mm//2, (mm%2)*256]
                    nc.tensor.matmul(ps[:, mm // 2, (mm % 2) * 256:(mm % 2) * 256 + 256],
                                     kf(8 + mm), qf(2), start=True, stop=True,
                                     skip_group_check=True, **kq)
                nc.scalar.activation(pt[:, :, :], ps[:, :, :],
                                     AF.Exp, scale=1.0 / 8192.0)
                pt4 = pt[:].rearrange("p r (hh f) -> p (r hh) f", hh=2)
                nc.vector.tensor_tensor(pt4[:, :, 0:128], pt4[:, :, 0:128],
                                        mask8_t[:, :, :], ALU.mult)
                for m2 in range(2):
                    nc.tensor.matmul(
                        po[:, 256:TQ],
                        V_f8[:, 8 + 2 * m2:8 + 2 * m2 + 2, h, :],
                        pt[:, m2, :].rearrange("p (two f) -> p two f", two=2),
                        start=False, stop=False, perf_mode=DR,
                        skip_group_check=True)
                ps = psS.tile([128, 2, TQ], F32, tag='score')
                pt = pbpool.tile([128, 2, TQ], F8, tag='probs')
                for mm in range(4):   # quad 3: block mm at [0, mm*128]
                    nc.tensor.matmul(ps[:, 0, mm * 128:mm * 128 + 128],
                                     kf(12 + mm), qf(3), start=True, stop=True,
                                     skip_group_check=True, **kq)
                nc.scalar.activation(pt[:, 0, :], ps[:, 0, :],
                                     AF.Exp, scale=1.0 / 8192.0)
                pt3 = pt[:, 0, :].rearrange("p (m f) -> p m f", f=128)
                nc.gpsimd.tensor_tensor(pt3[:], pt3[:], mask8_t[:, :, :],
                                        ALU.mult)
                for m2 in range(2):
                    nc.tensor.matmul(
                        po[:, 384:TQ],
                        V_f8[:, 12 + 2 * m2:12 + 2 * m2 + 2, h, :],
                        pt[:, 0, 256 * m2:256 * m2 + 256].rearrange(
                            "p (two f) -> p two f", two=2),
                        start=False, stop=(m2 == 1), perf_mode=DR,
                        skip_group_check=True)
                rec = rpool.tile([1, TQ], F32, tag='recip')
                nc.vector.reciprocal(rec[:], po[64:65, :])
                rec64 = rpool.tile([64, TQ], F32, tag='recip64')
                nc.gpsimd.partition_broadcast(rec64[:], rec[0:1, :])
                nc.vector.tensor_tensor(attnT[hb:hb + 64, hc, :], po[0:64, :],
                                        rec64[:], ALU.mult)

            def prefetch_cd():
                # Prefetch phase-C/D weights + residual during the attention
                # DMA-idle window. A tiny Pool write into each destination
                # (sourced from a mid-phase-A K evac) gives every DMA a WAR
                # dependency so the 9MB of prefetch traffic doesn't starve
                # phase A's own loads. Transfers issue on the idle SP queue.
                gate_src = K_f8[0:1, 4, 0:8]

                def gated_dma(dst_small, dst, src):
                    nc.gpsimd.tensor_copy(dst_small, gate_src)
                    nc.sync.dma_start(dst, src)

                gated_dma(xqb_t[0:1, 0, 0:8], xqb_t[:], xqb_r[:])
                gated_dma(wproj_sb[0:1, 0, 0, 0:8], wproj_sb[:], wproj_d[:])
                for mt4 in range(8):
                    gated_dma(wfc_sb[0:1, mt4 * 4, 0, 0:8],
                              wfc_sb[:, mt4 * 4:(mt4 + 1) * 4],
                              wfc_d[:, mt4 * 4:(mt4 + 1) * 4])
                for mt in range(8):
                    gated_dma(wfc2_sb[0:1, mt, 0, 0:8], wfc2_sb[:, mt],
                              wfc2_d[:, mt])

            # Interleave: after parts 0..15, K mt0-3 and the n2=0 V half are
            # done, which is everything heads 0..7 read besides late quads'
            # K columns (mt pairs are per head-group t4: heads 0-3 -> mt 0,1;
            # 4-7 -> mt 2,3). Early heads fill the Act trough at the A/B seam.
            heads = list(range(H if 'B' in phases else 0))

            def pop_heads(n, **kw):
                for _ in range(min(n, len(heads))):
                    head(heads.pop(0), **kw)

            # A head may only be emitted once every tile it reads has been
            # emitted (readers emitted before writers see stale memory):
            # heads 0-7 need K mt 0-3 (parts 0-15) and the full n2=0 V half
            # (parts 0-15); heads 8-15 need the n2=1 V half (parts 16-31).
            for i in range(16):
                k_part(i)
                v_part(i)
            pop_heads(4)
            for i in range(16, 20):
                k_part(i)
                v_part(i)
            pop_heads(2)
            for i in range(20, 24):
                k_part(i)
                v_part(i)
            prefetch_cd()
            pop_heads(2)
            for i in range(24, 32):
                k_part(i)
                v_part(i)
            if debug_taps:
                nc.sync.dma_start(taps['tap_hT'][:], hT[:])
                nc.sync.dma_start(taps['tap_hQ'][:], hQ[:])
        # Late heads get a deeper score-buffer rotation (3 tiles / 6 banks)
        # in the banks phase A's psA pool just released, keeping exp fed.
        with (
            tc.tile_pool(name='psS2', bufs=3, space='PSUM') as psS2,
            tc.tile_pool(name='psO2', bufs=2, space='PSUM') as psO2,
        ):
            pop_heads(len(heads), psS=psS2, psO=psO2)
            if debug_taps:
                nc.sync.dma_start(taps['tap_Q'][:], Q_f8[:])
                nc.sync.dma_start(taps['tap_K'][:], K_f8[:])
                nc.sync.dma_start(taps['tap_V'][:], V_f8[:])
        es_kqv.close()

        # x2T/h2T live through phases C+D
        es_mlp = ExitStack()
        mpool = es_mlp.enter_context(tc.tile_pool(name='mlp', bufs=1))
        x2T = mpool.tile([128, 8, TQ], F32)
        h2T = mpool.tile([128, 8, TQ], F8)

        # ======== Phases C+D in one scope ====
        with (
            tc.tile_pool(name='stageC', bufs=3) as scpool,
            tc.tile_pool(name='gT_pool', bufs=1) as gpool,
            tc.tile_pool(name='psC', bufs=4, space='PSUM') as psC,
        ):
            for mt in range(8 if 'C' in phases else 0):
                ps = psC.tile([128, TQ], F32)
                for kp in range(4):
                    nc.tensor.matmul(ps[:], wproj_sb[:, mt, 2 * kp:2 * kp + 2, :],
                                     attnT[:, 2 * kp:2 * kp + 2, :],
                                     start=(kp == 0), stop=(kp == 3), perf_mode=DR)
                # x2 = psum/1024 + (x + b_proj_eff)
                nc.vector.scalar_tensor_tensor(x2T[:, mt, :], ps[:], 1.0 / 1024.0,
                                               xqb_t[:, mt, :], ALU.mult, ALU.add)
                nc.scalar.activation(h2T[:, mt, :], x2T[:, mt, :], AF.Tanh,
                                     scale=alpha_t[:, 0:1])

            # ================= Phase D: MLP =================
            gT = gpool.tile([128, 32, TQ], F8)
            for mt in range(32 if 'D' in phases else 0):
                ps = psC.tile([128, TQ], F32)
                for kp in range(4):
                    nc.tensor.matmul(ps[:], wfc_sb[:, mt, 2 * kp:2 * kp + 2, :],
                                     h2T[:, 2 * kp:2 * kp + 2, :],
                                     start=(kp == 0), stop=(kp == 3), perf_mode=DR)
                if gelu_sigmoid:
                    zt = scpool.tile([128, TQ], F32, tag='gelu_z')
                    nc.vector.tensor_scalar(zt[:], ps[:], 1.0 / SW,
                                            bfc_t[:, mt:mt + 1], ALU.mult, ALU.add)
                    sg = scpool.tile([128, TQ], F32, tag='gelu_s')
                    nc.scalar.activation(sg[:], zt[:], AF.Sigmoid, scale=1.702)
                    nc.vector.tensor_tensor(gT[:, mt, :], zt[:], sg[:], ALU.mult)
                else:
                    nc.scalar.activation(gT[:, mt, :], ps[:], AF.Gelu,
                                         bias=bfc_t[:, mt:mt + 1], scale=1.0 / SW)

            # fc2 in two groups of 4 output tiles, kp-outer: each kp step
            # consumes gelu outputs as they land instead of serializing the
            # whole 16-step accumulation after the last gelu.
            if 'D' in phases:
                for g4 in range(2):
                    pss = [psC.tile([128, TQ], F32, tag=f'fc2_{m}', bufs=1,
                                    name=f'ps_fc2_{g4}_{m}')
                           for m in range(4)]
                    for kp in range(16):
                        for m in range(4):
                            nc.tensor.matmul(
                                pss[m][:],
                                wfc2_sb[:, g4 * 4 + m, 2 * kp:2 * kp + 2, :],
                                gT[:, 2 * kp:2 * kp + 2, :],
                                start=(kp == 0), stop=(kp == 15), perf_mode=DR)
                    for m in range(4):
                        mt = g4 * 4 + m
                        tmp = scpool.tile([128, TQ], F32, tag='bias2')
                        nc.vector.tensor_scalar(tmp[:], pss[m][:], 1.0 / SW,
                                                bfc2_t[:, mt:mt + 1], ALU.mult,
                                                ALU.add)
                        yt = scpool.tile([128, TQ], F32, tag='yout')
                        nc.vector.tensor_tensor(yt[:], tmp[:], x2T[:, mt, :],
                                                ALU.add)
                        nc.sync.dma_start(yT_r[:, mt, :], yt[:])
            if debug_taps:
                nc.sync.dma_start(taps['tap_attnT'][:], attnT[:])
                nc.sync.dma_start(taps['tap_x2'][:], x2T[:])
                nc.sync.dma_start(taps['tap_gT'][:], gT[:])
        es_mlp.close()

    nc.finalize()
    return nc


def _prep_inputs(x, alpha, gamma, beta, w_attn, b_attn, w_proj, b_proj,
                 w_fc, b_fc, w_fc2, b_fc2):
    f = np.float32
    f8 = ml_dtypes.float8_e4m3

    def tile_w(w, n_mt):
        # [K, M] -> [128, mt, kt, 128]: element [p, mt, kt, c] = w[kt*128+p, mt*128+c]
        kk, mm = w.shape
        return np.ascontiguousarray(
            w.reshape(kk // 128, 128, n_mt, 128).transpose(1, 2, 0, 3).astype(f8))

    # Fold DyT's gamma/beta into the consuming weights:
    #   w.T @ (g*t + b) = (g[:,None]*w).T @ t + (w.T @ b)
    g64 = np.asarray(gamma, np.float64)
    b64 = np.asarray(beta, np.float64)
    w64 = np.asarray(w_attn, np.float64)
    wp64 = np.asarray(w_proj, np.float64)
    wfc64 = np.asarray(w_fc, np.float64)
    wfc264 = np.asarray(w_fc2, np.float64)
    wq64, wk64, wv64 = w64[:, :C], w64[:, C:2 * C], w64[:, 2 * C:]
    bq_e = np.asarray(b_attn[:C], np.float64) + wq64.T @ b64
    bk_e = np.asarray(b_attn[C:2 * C], np.float64) + wk64.T @ b64
    bv_e = np.asarray(b_attn[2 * C:], np.float64) + wv64.T @ b64
    bfc_e = np.asarray(b_fc, np.float64) + wfc64.T @ b64
    # v-bias rides through attention (sum(p)=1) -> fold through w_proj
    bproj_e = np.asarray(b_proj, np.float64) + bv_e @ wp64

    # Column permutation for the scores-DoubleRow layout: m-tile mt=2t+i,
    # col c'=32c+r  <->  original col 64*(4t+c) + 32i + r  (head 4t+c, d=32i+r)
    mt_i = np.arange(8)
    cp = np.arange(128)
    tg = mt_i[:, None] // 2
    ig = mt_i[:, None] % 2
    cg = cp[None, :] // 32
    rg = cp[None, :] % 32
    perm = (256 * tg + 64 * cg + 32 * ig + rg).reshape(-1)

    wq_p = (SW * wq64 * g64[:, None])[:, perm]
    wk_p = (SW * wk64 * g64[:, None])[:, perm]
    bq_p = (SW * bq_e)[perm]
    bk_p = (SW * bk_e)[perm]

    wv = np.ascontiguousarray(
        (SW * wv64 * g64[:, None]).reshape(8, 128, C).transpose(1, 0, 2).astype(f8))
    bq = np.ascontiguousarray(bq_p.reshape(8, 128).T.astype(f))
    bk = np.ascontiguousarray(bk_p.reshape(8, 128).T.astype(f))
    bfc = np.ascontiguousarray(bfc_e.reshape(32, 128).T.astype(f))
    bfc2 = np.ascontiguousarray(
        np.asarray(b_fc2, np.float64).reshape(8, 128).T.astype(f))
    alpha_b = np.full((128, 1), float(np.asarray(alpha).reshape(-1)[0]), f)
    ones_f8 = np.ones((128, 16), f8)

    shared = dict(wq=tile_w(wq_p, 8), wk=tile_w(wk_p, 8), wv=wv,
                  wproj=tile_w(SW * wp64, 8),
                  wfc=tile_w(SW * wfc64 * g64[:, None], 32),
                  wfc2=tile_w(SW * wfc264, 8),
                  bq=bq, bk=bk, bfc=bfc, bfc2=bfc2,
                  alpha_b=alpha_b, ones_f8=ones_f8)

    in_maps = []
    for c in range(8):
        b, j = c // 4, c % 4
        xbT = np.asarray(x[b], f).T                       # [C, T] natural order
        xT = np.ascontiguousarray(xbT.astype(ml_dtypes.bfloat16))
        xQ = np.ascontiguousarray(xbT[:, j::4].astype(ml_dtypes.bfloat16))
        xqb = np.ascontiguousarray(
            (np.asarray(x[b, j::4, :], np.float64).T + bproj_e[:, None]).astype(f))
        # mask8[p, m, i] = keep iff key 128m+p <= query 4i+j (within a quad)
        r = np.arange(128)[:, None, None]
        mm = np.arange(4)[None, :, None]
        ii = np.arange(128)[None, None, :]
        mask8 = np.where(128 * mm + r <= 4 * ii + j, 1.0, 0.0).astype(f8)
        in_maps.append(dict(shared, xT=xT, xQ=xQ, xqb=xqb, mask8=mask8))
    return in_maps


def kernel(**inputs):
    if 'nc' not in _CACHE:
        _CACHE['nc'] = _build()
    nc = _CACHE['nc']
    in_maps = _prep_inputs(**inputs)
    res = run_bass_kernel_spmd(nc, in_maps, core_ids=list(range(8)))
    out = np.zeros((2, T, C), np.float32)
    for c in range(8):
        b, j = c // 4, c % 4
        out[b, j::4, :] = res.results[c]['yT'].T
    return out
